# revision 27
# baseline (speedup 1.0000x reference)
"""Trainium2 Bass kernel for nn_DGN (3x NNConv GNN + all-pairs L1 CBT).

Strategy (8 NeuronCores, SPMD):
  - Edges sorted by (dst, src), sharded so core c owns destination nodes
    [256c, 256(c+1)) as two 128-node chunks; per-(core,chunk) edge lists are
    padded to a uniform tile count so a single SPMD program serves all cores.
  - Per 128-edge tile: PE computes the edge-MLP z = ea' @ W' (bias folded via
    a ones-row), DVE/ACT fuse relu+multiply-by-gathered-source-features, and
    PE scatter-matmuls (one-hot lhsT) accumulate the per-node mean and the
    i-contraction in a wide [n, out*in] PSUM accumulator, reduced at chunk
    end.  Root-weight term is injected as one extra matmul into its own PSUM.
  - Source-feature gather runs as a SWDGE dma_gather straight from the
    AllGather output: h slices are stored into 128-wide (256B-aligned) rows
    so the collective result doubles as the gather table, and the next
    layer's h-independent edge-MLP tiles (z matmul + relu) are emitted
    before each collective so PE/ACT hide its ~50us latency.
  - CBT: per 4-row batch of local output rows, one DVE tensor_scalar
    (subtract, max 0) against a 4x-replicated transposed-h tile produces
    relu(h[j,k]-h[i,k]) for all j,k; a selection matmul on PE sums over k
    into a [128, 2048] PSUM block accumulated over 32 batches, using
    sum|d| = 2*sum relu(d) - (R_j - R_i).

Perf notes (axon tunnel, 2026-08-08): per-exec wall time is dominated by a
~75 ms PJRT-over-axon dispatch floor plus input upload (~6 ms per array +
~160 MB/s).  So all large constant tables (the one-hot scatter matrix S,
selection/identity matrices, replicated gather indices) are built on-device
from a few KB of packed indices, inputs ship as three per-dtype blobs, the
zero output buffers are never uploaded (the custom call allocates outputs
fresh; every element of d_out is written), and the output is bf16.
"""

import numpy as np
import ml_dtypes

import concourse.bass as bass
import concourse.bacc as bacc
import concourse.tile as tile
import concourse.mybir as mybir

BF16 = mybir.dt.bfloat16
F32 = mybir.dt.float32
I16 = mybir.dt.int16

N = 2048
E = 65536
NV = 6
C = 32
NCORES = 8
NPC = N // NCORES      # nodes per core = 256
CHUNK = 128            # node chunk (PSUM partition dim)
Op = mybir.AluOpType

_RUNNER_CACHE = {}


# --------------------------------------------------------------------------
# host-side prep
# --------------------------------------------------------------------------

def _bf16(a):
    return np.asarray(a, dtype=np.float32).astype(ml_dtypes.bfloat16)


def _permute_w(Wf, b):
    """[NV, in*out] + [in*out] -> [7, in*out] with columns re-ordered from
    (i-major) i*out+o to (o-major) o*in+i, bias folded as last row."""
    in_c = Wf.shape[1] // C
    Wb = np.concatenate([Wf, b[None, :]], axis=0)  # [7, in*out]
    cols = Wb.reshape(NV + 1, in_c, C)             # [7, i, o]
    return np.transpose(cols, (0, 2, 1)).reshape(NV + 1, in_c * C)  # (o, i)


def _layout(TC):
    """Byte offsets of every tensor inside the three per-dtype blobs."""
    Tt = 2 * TC
    Ep = Tt * 128
    bb = dict()  # bf16 blob: name -> (offset_elems, shape)
    off = 0
    for name, shape in [("eaT7", (7, Ep)), ("W1p", (7, C)),
                        ("W2p", (7, C * C)), ("W3p", (7, C * C))]:
        bb[name] = (off, shape)
        off += shape[0] * shape[1]
    nb_b = off
    bf = dict()  # f32 blob
    off = 0
    for name, shape in [("xsrc", (128, Tt)), ("xT2", (2, NPC)),
                        ("invd", (128, 2)), ("r1", (2, C)),
                        ("r2", (C + 1, C)), ("r3", (C + 1, C)),
                        ("dstloc", (128, Tt)), ("colsel", (128, 1))]:
        bf[name] = (off, shape)
        off += shape[0] * shape[1]
    nb_f = off
    bi = dict()  # i16 blob
    bi["idx"] = (0, (16, Tt * 8))
    nb_i = 16 * Tt * 8
    return (bb, nb_b), (bf, nb_f), (bi, nb_i)


def _prep(x, edge_attr, edge_index, W1, b1, root1, bias1, W2, b2, root2,
          bias2, W3, b3, root3, bias3):
    src = np.asarray(edge_index[0], dtype=np.int64)
    dst = np.asarray(edge_index[1], dtype=np.int64)
    x = np.asarray(x, dtype=np.float32)
    ea = np.asarray(edge_attr, dtype=np.float32)

    deg = np.bincount(dst, minlength=N).astype(np.float64)
    inv_deg = (1.0 / np.maximum(deg, 1.0)).astype(np.float32)

    order = np.lexsort((src, dst))
    s_srt, d_srt = src[order], dst[order]

    # per 128-dst-chunk edge index lists (into the original edge arrays)
    groups = []
    for g in range(N // CHUNK):          # 16 chunks
        sel = order[(d_srt >= g * CHUNK) & (d_srt < (g + 1) * CHUNK)]
        groups.append(sel)
    TC = max(1, max((len(g) + 127) // 128 for g in groups))
    Tt = 2 * TC
    Ep = Tt * 128

    (bb_l, nb_b), (bf_l, nb_f), (bi_l, nb_i) = _layout(TC)

    shared_b = {
        "W1p": _bf16(np.concatenate([W1, b1[None, :]], 0)),
        "W2p": _bf16(_permute_w(W2, b2)),
        "W3p": _bf16(_permute_w(W3, b3)),
    }
    shared_f = {
        "r1": np.stack([root1[0], bias1], 0).astype(np.float32),
        "r2": np.concatenate([root2, bias2[None, :]], 0).astype(np.float32),
        "r3": np.concatenate([root3, bias3[None, :]], 0).astype(np.float32),
        "colsel": (124 + np.arange(128) // 32).astype(np.float32)[:, None],
    }

    in_maps = []
    for c in range(NCORES):
        eaT7 = np.zeros((7, Ep), np.float32)
        xsrc = np.zeros((128, Tt), np.float32)
        dstloc = np.full((128, Tt), 300.0, np.float32)  # 300 = no-match pad
        idx16 = np.zeros((16, Tt * 8), np.int16)
        for ch in range(2):
            g = groups[2 * c + ch]
            n = len(g)
            base = ch * TC * 128
            eaT7[:NV, base:base + n] = ea[g].T
            eaT7[NV, base:base + n] = 1.0
            gs = src[g]
            gd = dst[g]
            # edge-slot (p, t) layout: slot j of chunk ch -> p=j%128, t=j//128
            p = np.arange(n) % 128
            t = ch * TC + np.arange(n) // 128
            xsrc[p, t] = x[gs, 0]
            dstloc[p, t] = (gd - (2 * c + ch) * CHUNK).astype(np.float32)
            # gather indices, wrapped: idx j -> [j%16, j//16]
            ids = np.zeros(TC * 128, np.int16)
            ids[:n] = gs.astype(np.int16)
            idx16[:, ch * TC * 8:(ch + 1) * TC * 8] = \
                ids.reshape(TC * 8, 16).T
        xT2 = np.zeros((2, NPC), np.float32)
        xT2[0] = x[c * NPC:(c + 1) * NPC, 0]
        xT2[1] = 1.0
        invd = inv_deg[c * NPC:(c + 1) * NPC].reshape(2, 128).T.copy()

        per_b = {"eaT7": _bf16(eaT7), **shared_b}
        per_f = {"xsrc": xsrc, "xT2": xT2, "invd": invd,
                 "dstloc": dstloc, **shared_f}
        blob_b = np.zeros((1, nb_b), ml_dtypes.bfloat16)
        for name, (off, shape) in bb_l.items():
            blob_b[0, off:off + shape[0] * shape[1]] = \
                np.ascontiguousarray(per_b[name]).reshape(-1)
        blob_f = np.zeros((1, nb_f), np.float32)
        for name, (off, shape) in bf_l.items():
            blob_f[0, off:off + shape[0] * shape[1]] = \
                np.ascontiguousarray(per_f[name]).reshape(-1)
        blob_i = np.zeros((1, nb_i), np.int16)
        off, shape = bi_l["idx"]
        blob_i[0, off:off + shape[0] * shape[1]] = idx16.reshape(-1)
        in_maps.append({"bb": blob_b, "bf": blob_f, "bi": blob_i})
    return TC, in_maps


# --------------------------------------------------------------------------
# device program
# --------------------------------------------------------------------------

def build_program(TC, reps=1, ablate=()):
    ablate = set(ablate)
    Tt = 2 * TC
    Ep = Tt * 128
    (bb_l, nb_b), (bf_l, nb_f), (bi_l, nb_i) = _layout(TC)
    nc = bacc.Bacc("TRN2", target_bir_lowering=False, debug=False,
                   num_devices=NCORES)

    d_bb = nc.dram_tensor("bb", [1, nb_b], BF16, kind="ExternalInput")
    d_bf = nc.dram_tensor("bf", [1, nb_f], F32, kind="ExternalInput")
    d_bi = nc.dram_tensor("bi", [1, nb_i], I16, kind="ExternalInput")
    d_out = nc.dram_tensor("out", [NPC, N], BF16, kind="ExternalOutput")

    # layers 0/1: 128-wide rows so the AllGather output doubles as the
    # 256B-aligned dma_gather table (cols C:128 are never-read garbage)
    d_hsl = [nc.dram_tensor(f"hsl{l}", [NPC, 128], BF16) for l in range(2)]
    d_hsl.append(nc.dram_tensor("hsl2", [NPC, C], F32))
    d_hall = [nc.dram_tensor(f"hall{l}", [N, 128], BF16, addr_space="Shared")
              for l in range(2)]
    d_hall.append(nc.dram_tensor("hall2", [N, C], F32, addr_space="Shared"))

    RG = [list(range(NCORES))]

    def blob_ap(dram, layout, name):
        off, shape = layout[name]
        n = shape[0] * shape[1]
        return dram.ap()[0:1, off:off + n].rearrange(
            "a (p m) -> (a p) m", p=shape[0])

    with tile.TileContext(nc) as tc:
      for _rep in range(reps):
        with (
            tc.tile_pool(name="const", bufs=1) as cp,
            tc.tile_pool(name="hgp", bufs=2) as hgp,
            tc.tile_pool(name="msgp", bufs=6) as msgp,
            tc.tile_pool(name="wrp", bufs=TC + 2) as wrp,
            tc.tile_pool(name="tp", bufs=4) as tpp,
            tc.tile_pool(name="hcp", bufs=6) as hcp,
            tc.tile_pool(name="smf", bufs=6) as smf,
            tc.tile_pool(name="zp", bufs=2, space="PSUM") as zp,
            tc.tile_pool(name="aggp", bufs=1, space="PSUM") as aggp,
            tc.tile_pool(name="smp", bufs=1, space="PSUM") as smp,
        ):
            def bload(dram, layout, name, dtype, tag, shape=None):
                off, tshape = layout[name]
                shape = shape or tshape
                t = cp.tile(list(shape), dtype, tag=tag)
                nc.sync.dma_start(
                    out=t[0:tshape[0], :], in_=blob_ap(dram, layout, name))
                return t

            ea_sb = bload(d_bb, bb_l, "eaT7", BF16, "ea")
            w1_sb = bload(d_bb, bb_l, "W1p", BF16, "w1")
            w2_sb = bload(d_bb, bb_l, "W2p", BF16, "w2")
            w3_sb = bload(d_bb, bb_l, "W3p", BF16, "w3")
            xs_sb = bload(d_bf, bf_l, "xsrc", F32, "xs")
            xT2_sb = bload(d_bf, bf_l, "xT2", F32, "xT2")
            invd_sb = bload(d_bf, bf_l, "invd", F32, "invd")
            r1_sb = bload(d_bf, bf_l, "r1", F32, "r1")
            r2_sb = bload(d_bf, bf_l, "r2", F32, "r2")
            r3_sb = bload(d_bf, bf_l, "r3", F32, "r3")
            dst_sb = bload(d_bf, bf_l, "dstloc", F32, "dst")
            csel_sb = bload(d_bf, bf_l, "colsel", F32, "csel")
            # gather idx: load wrapped [16, Tt*8] then replicate to 128 rows
            ix_sb = cp.tile([128, Tt * 8], I16, tag="ix")
            nc.sync.dma_start(out=ix_sb[0:16, :],
                              in_=blob_ap(d_bi, bi_l, "idx"))
            nc.sync.dma_start(out=ix_sb[16:32, :], in_=ix_sb[0:16, :])
            nc.sync.dma_start(out=ix_sb[32:64, :], in_=ix_sb[0:32, :])
            nc.sync.dma_start(out=ix_sb[64:128, :], in_=ix_sb[0:64, :])

            # --------- on-device constant tables ---------
            iota128 = cp.tile([128, 128], F32, tag="iota128")
            nc.gpsimd.iota(iota128[:], pattern=[[1, 128]], base=0,
                           channel_multiplier=0,
                           allow_small_or_imprecise_dtypes=True)
            iotasel = cp.tile([128, 252], F32, tag="iotasel")
            nc.gpsimd.iota(iotasel[:], pattern=[[1, 252]], base=0,
                           channel_multiplier=0,
                           allow_small_or_imprecise_dtypes=True)
            idjp = cp.tile([128, 128], F32, tag="idjp")
            nc.gpsimd.iota(idjp[:], pattern=[[1, 128]], base=0,
                           channel_multiplier=-1,
                           allow_small_or_imprecise_dtypes=True)
            id32_sb = cp.tile([128, 128], F32, tag="id32")
            nc.vector.tensor_scalar(id32_sb[:], idjp[:], 0.0, None,
                                    Op.is_equal)
            sel_sb = cp.tile([128, 252], BF16, tag="sel")
            nc.vector.tensor_scalar(sel_sb[:], iotasel[:],
                                    csel_sb[:, 0:1], None, Op.is_equal)
            seln_sb = cp.tile([128, 128], F32, tag="seln")
            nc.vector.memset(seln_sb[0:32, :], -0.5)
            for qp in range(1, 4):
                nc.vector.memset(seln_sb[32 * qp:32 * (qp + 1), :], 0.0)
            # scatter one-hot S[e, n] = (dstloc[e,t] == n), built per tile
            S_sb = cp.tile([128, Ep], BF16, tag="S")
            for t in range(Tt):
                nc.vector.tensor_scalar(
                    S_sb[:, t * 128:(t + 1) * 128], iota128[:],
                    dst_sb[:, t:t + 1], None, Op.is_equal)

            # deterministic (never-read) padding columns of the gather tables
            zpad = cp.tile([128, 128 - C], BF16, tag="zpad")
            nc.vector.memset(zpad[:], 0.0)
            for l in range(2):
                for ch in range(2):
                    nc.sync.dma_start(
                        out=d_hsl[l][ch * 128:(ch + 1) * 128, C:128],
                        in_=zpad[:])

            hT1 = cp.tile([C + 1, NPC], F32, tag="hT1")
            hT2 = cp.tile([C + 1, NPC], F32, tag="hT2")
            hT3 = cp.tile([C, NPC], F32, tag="hT3")
            Rloc = cp.tile([128, 2], F32, tag="Rloc")
            nc.vector.memset(hT1[C:C + 1, :], 1.0)
            nc.vector.memset(hT2[C:C + 1, :], 1.0)

            # ---------------- layer 1 (in_c = 1) ----------------
            for ch in range(2):
                agg = aggp.tile([128, C], F32, tag="aggw")
                for t in range(TC):
                    gt = ch * TC + t
                    z1 = zp.tile([128, C], F32, tag="z")
                    nc.tensor.matmul(z1[:], ea_sb[:, gt * 128:(gt + 1) * 128],
                                     w1_sb[:], start=True, stop=True)
                    msg = msgp.tile([128, C], BF16, tag="msg")
                    nc.vector.tensor_scalar(
                        msg[:], z1[:], 0.0, xs_sb[:, gt:gt + 1],
                        Op.max, Op.mult)
                    nc.tensor.matmul(agg[:], S_sb[:, gt * 128:(gt + 1) * 128],
                                     msg[:], start=(t == 0), stop=(t == TC - 1))
                rtp = smp.tile([128, C], F32, tag="root")
                nc.tensor.matmul(rtp[:], xT2_sb[:, ch * 128:(ch + 1) * 128],
                                 r1_sb[:], start=True, stop=True)
                sm = smf.tile([128, C], F32, tag="sm")
                nc.vector.tensor_scalar(sm[:], agg[:],
                                        invd_sb[:, ch:ch + 1], None, Op.mult)
                hf_c = hcp.tile([128, C], F32, tag="hf")
                nc.vector.tensor_tensor(hf_c[:], sm[:], rtp[:], Op.add)
                nc.vector.tensor_scalar(hf_c[:], hf_c[:], 0.0, None, Op.max)
                h_c = hcp.tile([128, C], BF16, tag="hc")
                nc.scalar.copy(h_c[:], hf_c[:])
                tp = smp.tile([32, 128], F32, tag="tp")
                nc.tensor.transpose(tp[:], hf_c[:], id32_sb[:])
                nc.scalar.copy(hT1[0:C, ch * 128:(ch + 1) * 128], tp[:])
                nc.sync.dma_start(out=d_hsl[0][ch * 128:(ch + 1) * 128, 0:C],
                                  in_=h_c[:])
            def allgather(l):
                if "cc" in ablate:
                    nc.sync.dma_start(out=d_hall[l][0:NPC, :],
                                      in_=d_hsl[l].ap())
                else:
                    nc.gpsimd.collective_compute(
                        "AllGather", Op.bypass, replica_groups=RG,
                        ins=[d_hsl[l].ap()], outs=[d_hall[l].ap()])

            def phase_a(w_sb):
                """h-independent edge-MLP for chunk 0 of the next layer;
                emitted before the AllGather so PE/ACT overlap it."""
                tiles = []
                for t in range(TC):
                    z = zp.tile([128, C * C], F32, tag="z")
                    for q in range(2):
                        nc.tensor.matmul(
                            z[:, q * 512:(q + 1) * 512],
                            ea_sb[:, t * 128:(t + 1) * 128],
                            w_sb[:, q * 512:(q + 1) * 512],
                            start=True, stop=True)
                    wr = wrp.tile([128, C * C], BF16, tag="wr")
                    if t % 4 == 1:
                        nc.vector.tensor_scalar(wr[:], z[:], 0.0, None, Op.max)
                    else:
                        nc.scalar.activation(
                            wr[:], z[:], mybir.ActivationFunctionType.Relu)
                    tiles.append(wr)
                return tiles

            wrA = phase_a(w2_sb)
            allgather(0)

            # ---------------- layers 2 and 3 ----------------
            for li, (w_sb, r_sb, hTprev, hTcur) in enumerate(
                    [(w2_sb, r2_sb, hT1, hT2), (w3_sb, r3_sb, hT2, hT3)]):
                tab = d_hall[li]
                for ch in range(2):
                    hg = hgp.tile([128, TC, 128], BF16, tag="hg")
                    if "gather" in ablate:
                        # timing ablation: same bytes via plain DMA
                        done = 0
                        while done < TC:
                            nt = min(16, TC - done)
                            nc.sync.dma_start(
                                out=hg[:, done:done + nt, :],
                                in_=tab.ap()[0:nt * 128, :].rearrange(
                                    "(t p) e -> p t e", p=128))
                            done += nt
                    else:
                        # <=512 indices per dma_gather: one 4224-idx gather
                        # overflows the SWDGE descriptor ring (hang).
                        for g in range((TC * 128 + 511) // 512):
                            n_idx = min(512, TC * 128 - g * 512)
                            base = ch * TC * 8 + g * 32
                            nc.gpsimd.dma_gather(
                                out_ap=hg[:, g * 4:g * 4 + (n_idx + 127) // 128, :],
                                in_ap=tab.ap(),
                                idxs_ap=ix_sb[:, base:base + (n_idx + 15) // 16],
                                num_idxs=n_idx, num_idxs_reg=n_idx,
                                elem_size=128)
                    aggw = aggp.tile([128, C * C], F32, tag="aggw")
                    for t in range(TC):
                        gt = ch * TC + t
                        if ch == 0:
                            wr = wrA[t]
                        else:
                            z = zp.tile([128, C * C], F32, tag="z")
                            for q in range(2):
                                nc.tensor.matmul(
                                    z[:, q * 512:(q + 1) * 512],
                                    ea_sb[:, gt * 128:(gt + 1) * 128],
                                    w_sb[:, q * 512:(q + 1) * 512],
                                    start=True, stop=True)
                            wr = wrp.tile([128, C * C], BF16, tag="wr")
                            if t % 4 == 1:
                                nc.vector.tensor_scalar(
                                    wr[:], z[:], 0.0, None, Op.max)
                            else:
                                nc.scalar.activation(
                                    wr[:], z[:],
                                    mybir.ActivationFunctionType.Relu)
                        tt = tpp.tile([128, C * C], BF16, tag="t")
                        t3 = tt[:].rearrange("p (o i) -> p o i", i=C)
                        hgb = hg[:, t:t + 1, 0:C].broadcast_to([128, C, C])
                        wr3 = wr[:].rearrange("p (o i) -> p o i", i=C)
                        nc.vector.tensor_tensor(t3, wr3, hgb, Op.mult)
                        for q in range(2):
                            nc.tensor.matmul(
                                aggw[:, q * 512:(q + 1) * 512],
                                S_sb[:, gt * 128:(gt + 1) * 128],
                                tt[:, q * 512:(q + 1) * 512],
                                start=(t == 0), stop=(t == TC - 1))
                    rtp = smp.tile([128, C], F32, tag="root")
                    nc.tensor.matmul(rtp[:],
                                     hTprev[:, ch * 128:(ch + 1) * 128],
                                     r_sb[:], start=True, stop=True)
                    red = smf.tile([128, C], F32, tag="red")
                    nc.vector.tensor_reduce(
                        red[:], aggw[:].rearrange("p (o i) -> p o i", i=C),
                        mybir.AxisListType.X, Op.add)
                    sm = smf.tile([128, C], F32, tag="sm")
                    nc.vector.tensor_scalar(sm[:], red[:],
                                            invd_sb[:, ch:ch + 1], None,
                                            Op.mult)
                    hf_c = hcp.tile([128, C], F32, tag="hf")
                    nc.vector.tensor_tensor(hf_c[:], sm[:], rtp[:], Op.add)
                    nc.vector.tensor_scalar(hf_c[:], hf_c[:], 0.0, None,
                                            Op.max)
                    if li == 1:
                        nc.vector.tensor_reduce(
                            Rloc[:, ch:ch + 1], hf_c[:],
                            mybir.AxisListType.X, Op.add)
                    tp = smp.tile([32, 128], F32, tag="tp")
                    nc.tensor.transpose(tp[:], hf_c[:], id32_sb[:])
                    nc.scalar.copy(hTcur[0:C, ch * 128:(ch + 1) * 128], tp[:])
                    if li == 0:
                        h_c = hcp.tile([128, C], BF16, tag="hc")
                        nc.scalar.copy(h_c[:], hf_c[:])
                        nc.sync.dma_start(
                            out=d_hsl[1][ch * 128:(ch + 1) * 128, 0:C],
                            in_=h_c[:])
                    else:
                        nc.sync.dma_start(
                            out=d_hsl[2][ch * 128:(ch + 1) * 128, :],
                            in_=hf_c[:])
                if li == 0:
                    wrA = phase_a(w3_sb)
                allgather(li + 1)

            # ---------------- CBT prep ----------------
            # f32 differencing: bf16 h here blows up per-entry relative
            # error on near-zero CBT entries (cancellation), keep f32
            hTrep = cp.tile([128, N], F32, tag="hTrep")
            h3f = cp.tile([128, 16, C], F32, tag="h3f")
            nc.sync.dma_start(
                out=h3f[:], in_=d_hall[2].ap().rearrange("(a b) c -> b a c",
                                                         a=16))
            for j in range(16):
                tp = smp.tile([32, 128], F32, tag="tp" if j % 2 else "root")
                nc.tensor.transpose(tp[:], h3f[:, j, :], id32_sb[:])
                nc.scalar.copy(hTrep[0:32, j * 128:(j + 1) * 128], tp[:])
            nc.sync.dma_start(out=hTrep[32:64, :], in_=hTrep[0:32, :])
            nc.sync.dma_start(out=hTrep[64:128, :], in_=hTrep[0:64, :])
            htcols = cp.tile([128, NPC // 4], F32, tag="htcols")
            htcolsn = cp.tile([128, NPC // 4], F32, tag="htcolsn")
            for s in range(4):
                nc.sync.dma_start(
                    out=htcols[s * 32:(s + 1) * 32, :],
                    in_=hT3[0:C, s::4])
            nc.vector.tensor_scalar(htcolsn[:], htcols[:], -1.0, None,
                                    Op.mult)

        # ---------------- CBT ----------------
        with (
            tc.tile_pool(name="ap_", bufs=3) as apool,
            tc.tile_pool(name="ob", bufs=4) as ob,
            tc.tile_pool(name="cbtp", bufs=2, space="PSUM") as cbtp,
        ):
            for ib in range(2):
                cps = cbtp.tile([128, N], F32, tag="cps")
                for b in range(32):
                    col = ib * 32 + b
                    a_b = apool.tile([128, N], BF16, tag="a")
                    if b % 7 >= 4:
                        nc.scalar.activation(
                            a_b[:], hTrep[:],
                            mybir.ActivationFunctionType.Relu,
                            bias=htcolsn[:, col:col + 1])
                    else:
                        nc.vector.tensor_scalar(
                            a_b[:], hTrep[:], htcols[:, col:col + 1], 0.0,
                            Op.subtract, Op.max)
                    lo = 124 - 4 * b
                    for q in range(4):
                        nc.tensor.matmul(
                            cps[:, q * 512:(q + 1) * 512],
                            sel_sb[:, lo:lo + 128],
                            a_b[:, q * 512:(q + 1) * 512],
                            start=(b == 0), stop=False)
                for q in range(4):
                    nc.tensor.matmul(
                        cps[:, q * 512:(q + 1) * 512], seln_sb[:],
                        hTrep[:, q * 512:(q + 1) * 512],
                        start=False, stop=True)
                for q in range(4):
                    ob_t = ob.tile([128, 512], BF16, tag="ob")
                    if q % 2 == 0:
                        nc.scalar.activation(
                            ob_t[:], cps[:, q * 512:(q + 1) * 512],
                            mybir.ActivationFunctionType.Identity,
                            bias=Rloc[:, ib:ib + 1], scale=2.0)
                    else:
                        nc.vector.tensor_scalar(
                            ob_t[:], cps[:, q * 512:(q + 1) * 512],
                            2.0, Rloc[:, ib:ib + 1], Op.mult, Op.add)
                    nc.sync.dma_start(
                        out=d_out[ib * 128:(ib + 1) * 128,
                                  q * 512:(q + 1) * 512],
                        in_=ob_t[:])

    nc.compile()
    return nc


# --------------------------------------------------------------------------
# runner: custom PJRT dispatch (no zero-output upload, 3 input arrays)
# --------------------------------------------------------------------------

class _Runner:
    def __init__(self, TC, reps=1):
        import jax
        from jax.sharding import Mesh, PartitionSpec
        from jax.experimental.shard_map import shard_map
        from concourse import bass2jax
        bass2jax.install_neuronx_cc_hook()

        self.jax = jax
        nc = build_program(TC, reps=reps)
        self.nc = nc
        partition_name = (nc.partition_id_tensor.name
                          if nc.partition_id_tensor else None)
        in_names, out_names, out_avals = [], [], []
        for alloc in nc.m.functions[0].allocations:
            if not isinstance(alloc, mybir.MemoryLocationSet):
                continue
            name = alloc.memorylocations[0].name
            if alloc.kind == "ExternalInput":
                if name != partition_name:
                    in_names.append(name)
            elif alloc.kind == "ExternalOutput":
                out_names.append(name)
                out_avals.append(jax.core.ShapedArray(
                    tuple(alloc.tensor_shape), mybir.dt.np(alloc.dtype)))
        self.in_names = in_names
        self.out_names = out_names
        all_in = list(in_names)
        if partition_name is not None:
            all_in.append(partition_name)

        def _body(*args):
            operands = list(args)
            if partition_name is not None:
                operands.append(bass2jax.partition_id_tensor())
            outs = bass2jax._bass_exec_p.bind(
                *operands, out_avals=tuple(out_avals),
                in_names=tuple(all_in), out_names=tuple(out_names),
                lowering_input_output_aliases=(),
                sim_require_finite=True, sim_require_nnan=True, nc=nc)
            return tuple(outs)

        devices = jax.devices()[:NCORES]
        mesh = Mesh(np.asarray(devices), ("core",))
        self.sharded = jax.jit(shard_map(
            _body, mesh=mesh,
            in_specs=(PartitionSpec("core"),) * len(in_names),
            out_specs=(PartitionSpec("core"),) * len(out_names),
            check_rep=False), keep_unused=True)

    def global_args(self, in_maps):
        return [np.concatenate([in_maps[c][nm] for c in range(NCORES)], 0)
                for nm in self.in_names]

    def __call__(self, in_maps):
        out = self.sharded(*self.global_args(in_maps))
        self.jax.block_until_ready(out)
        return np.asarray(out[0])


def _get_runner(TC):
    r = _RUNNER_CACHE.get(TC)
    if r is None:
        r = _Runner(TC)
        _RUNNER_CACHE[TC] = r
    return r


def kernel(**inputs):
    TC, in_maps = _prep(**inputs)
    runner = _get_runner(TC)
    out = runner(in_maps)                    # [NCORES*NPC, N] bf16
    return np.asarray(out, dtype=np.float32)


# revision 29
# speedup vs baseline: 1.0461x; 1.0461x over previous
"""Trainium2 Bass kernel for nn_DGN (3x NNConv GNN + all-pairs L1 CBT).

Strategy (8 NeuronCores, SPMD):
  - Edges sorted by (dst, src), sharded so core c owns destination nodes
    [256c, 256(c+1)) as two 128-node chunks; per-(core,chunk) edge lists are
    padded to a uniform tile count so a single SPMD program serves all cores.
  - Per 128-edge tile: PE computes the edge-MLP z = ea' @ W' (bias folded via
    a ones-row), DVE/ACT fuse relu+multiply-by-gathered-source-features, and
    PE scatter-matmuls (one-hot lhsT) accumulate the per-node mean and the
    i-contraction in a wide [n, out*in] PSUM accumulator, reduced at chunk
    end.  Root-weight term is injected as one extra matmul into its own PSUM.
  - Source-feature gather runs as a SWDGE dma_gather straight from the
    AllGather output: h slices are stored into 128-wide (256B-aligned) rows
    so the collective result doubles as the gather table, and the next
    layer's h-independent edge-MLP tiles (z matmul + relu) are emitted
    before each collective so PE/ACT hide its ~50us latency.
  - CBT: per 4-row batch of local output rows, one DVE tensor_scalar
    (subtract, max 0) against a 4x-replicated transposed-h tile produces
    relu(h[j,k]-h[i,k]) for all j,k; a selection matmul on PE sums over k
    into a [128, 2048] PSUM block accumulated over 32 batches, using
    sum|d| = 2*sum relu(d) - (R_j - R_i).

Perf notes (axon tunnel, 2026-08-08): per-exec wall time is dominated by a
~75 ms PJRT-over-axon dispatch floor plus input upload (~6 ms per array +
~160 MB/s).  So all large constant tables (the one-hot scatter matrix S,
selection/identity matrices, replicated gather indices) are built on-device
from a few KB of packed indices, inputs ship as three per-dtype blobs, the
zero output buffers are never uploaded (the custom call allocates outputs
fresh; every element of d_out is written), and the output is bf16.
"""

import numpy as np
import ml_dtypes

import concourse.bass as bass
import concourse.bacc as bacc
import concourse.tile as tile
import concourse.mybir as mybir

BF16 = mybir.dt.bfloat16
F32 = mybir.dt.float32
I16 = mybir.dt.int16

N = 2048
E = 65536
NV = 6
C = 32
NCORES = 8
NPC = N // NCORES      # nodes per core = 256
CHUNK = 128            # node chunk (PSUM partition dim)
Op = mybir.AluOpType

_RUNNER_CACHE = {}


# --------------------------------------------------------------------------
# host-side prep
# --------------------------------------------------------------------------

def _bf16(a):
    return np.asarray(a, dtype=np.float32).astype(ml_dtypes.bfloat16)


def _permute_w(Wf, b):
    """[NV, in*out] + [in*out] -> [7, in*out] with columns re-ordered from
    (i-major) i*out+o to (o-major) o*in+i, bias folded as last row."""
    in_c = Wf.shape[1] // C
    Wb = np.concatenate([Wf, b[None, :]], axis=0)  # [7, in*out]
    cols = Wb.reshape(NV + 1, in_c, C)             # [7, i, o]
    return np.transpose(cols, (0, 2, 1)).reshape(NV + 1, in_c * C)  # (o, i)


def _layout(TC):
    """Byte offsets of every tensor inside the three per-dtype blobs."""
    Tt = 2 * TC
    Ep = Tt * 128
    bb = dict()  # bf16 blob: name -> (offset_elems, shape)
    off = 0
    for name, shape in [("eaT7", (7, Ep)), ("W1p", (7, C)),
                        ("W2p", (7, C * C)), ("W3p", (7, C * C))]:
        bb[name] = (off, shape)
        off += shape[0] * shape[1]
    nb_b = off
    bf = dict()  # f32 blob
    off = 0
    for name, shape in [("xsrc", (128, Tt)), ("xT2", (2, NPC)),
                        ("invd", (128, 2)), ("r1", (2, C)),
                        ("r2", (C + 1, C)), ("r3", (C + 1, C)),
                        ("dstloc", (128, Tt)), ("colsel", (128, 1))]:
        bf[name] = (off, shape)
        off += shape[0] * shape[1]
    nb_f = off
    bi = dict()  # i16 blob
    bi["idx"] = (0, (16, Tt * 8))
    nb_i = 16 * Tt * 8
    return (bb, nb_b), (bf, nb_f), (bi, nb_i)


def _prep(x, edge_attr, edge_index, W1, b1, root1, bias1, W2, b2, root2,
          bias2, W3, b3, root3, bias3):
    src = np.asarray(edge_index[0], dtype=np.int64)
    dst = np.asarray(edge_index[1], dtype=np.int64)
    x = np.asarray(x, dtype=np.float32)
    ea = np.asarray(edge_attr, dtype=np.float32)

    deg = np.bincount(dst, minlength=N).astype(np.float64)
    inv_deg = (1.0 / np.maximum(deg, 1.0)).astype(np.float32)

    order = np.lexsort((src, dst))
    s_srt, d_srt = src[order], dst[order]

    # per 128-dst-chunk edge index lists (into the original edge arrays)
    groups = []
    for g in range(N // CHUNK):          # 16 chunks
        sel = order[(d_srt >= g * CHUNK) & (d_srt < (g + 1) * CHUNK)]
        groups.append(sel)
    TC = max(1, max((len(g) + 127) // 128 for g in groups))
    Tt = 2 * TC
    Ep = Tt * 128

    (bb_l, nb_b), (bf_l, nb_f), (bi_l, nb_i) = _layout(TC)

    shared_b = {
        "W1p": _bf16(np.concatenate([W1, b1[None, :]], 0)),
        "W2p": _bf16(_permute_w(W2, b2)),
        "W3p": _bf16(_permute_w(W3, b3)),
    }
    shared_f = {
        "r1": np.stack([root1[0], bias1], 0).astype(np.float32),
        "r2": np.concatenate([root2, bias2[None, :]], 0).astype(np.float32),
        "r3": np.concatenate([root3, bias3[None, :]], 0).astype(np.float32),
        "colsel": (124 + np.arange(128) // 32).astype(np.float32)[:, None],
    }

    in_maps = []
    for c in range(NCORES):
        eaT7 = np.zeros((7, Ep), np.float32)
        xsrc = np.zeros((128, Tt), np.float32)
        dstloc = np.full((128, Tt), 300.0, np.float32)  # 300 = no-match pad
        idx16 = np.zeros((16, Tt * 8), np.int16)
        for ch in range(2):
            g = groups[2 * c + ch]
            n = len(g)
            base = ch * TC * 128
            eaT7[:NV, base:base + n] = ea[g].T
            eaT7[NV, base:base + n] = 1.0
            gs = src[g]
            gd = dst[g]
            # edge-slot (p, t) layout: slot j of chunk ch -> p=j%128, t=j//128
            p = np.arange(n) % 128
            t = ch * TC + np.arange(n) // 128
            xsrc[p, t] = x[gs, 0]
            dstloc[p, t] = (gd - (2 * c + ch) * CHUNK).astype(np.float32)
            # gather indices, wrapped: idx j -> [j%16, j//16]
            ids = np.zeros(TC * 128, np.int16)
            ids[:n] = gs.astype(np.int16)
            idx16[:, ch * TC * 8:(ch + 1) * TC * 8] = \
                ids.reshape(TC * 8, 16).T
        xT2 = np.zeros((2, NPC), np.float32)
        xT2[0] = x[c * NPC:(c + 1) * NPC, 0]
        xT2[1] = 1.0
        invd = inv_deg[c * NPC:(c + 1) * NPC].reshape(2, 128).T.copy()

        per_b = {"eaT7": _bf16(eaT7), **shared_b}
        per_f = {"xsrc": xsrc, "xT2": xT2, "invd": invd,
                 "dstloc": dstloc, **shared_f}
        blob_b = np.zeros((1, nb_b), ml_dtypes.bfloat16)
        for name, (off, shape) in bb_l.items():
            blob_b[0, off:off + shape[0] * shape[1]] = \
                np.ascontiguousarray(per_b[name]).reshape(-1)
        blob_f = np.zeros((1, nb_f), np.float32)
        for name, (off, shape) in bf_l.items():
            blob_f[0, off:off + shape[0] * shape[1]] = \
                np.ascontiguousarray(per_f[name]).reshape(-1)
        blob_i = np.zeros((1, nb_i), np.int16)
        off, shape = bi_l["idx"]
        blob_i[0, off:off + shape[0] * shape[1]] = idx16.reshape(-1)
        in_maps.append({"bb": blob_b, "bf": blob_f, "bi": blob_i})
    return TC, in_maps


# --------------------------------------------------------------------------
# device program
# --------------------------------------------------------------------------

def build_program(TC, reps=1, ablate=()):
    ablate = set(ablate)
    Tt = 2 * TC
    Ep = Tt * 128
    (bb_l, nb_b), (bf_l, nb_f), (bi_l, nb_i) = _layout(TC)
    nc = bacc.Bacc("TRN2", target_bir_lowering=False, debug=False,
                   num_devices=NCORES)

    d_bb = nc.dram_tensor("bb", [1, nb_b], BF16, kind="ExternalInput")
    d_bf = nc.dram_tensor("bf", [1, nb_f], F32, kind="ExternalInput")
    d_bi = nc.dram_tensor("bi", [1, nb_i], I16, kind="ExternalInput")
    d_out = nc.dram_tensor("out", [NPC, N], BF16, kind="ExternalOutput")

    # layers 0/1: 128-wide rows so the AllGather output doubles as the
    # 256B-aligned dma_gather table (cols C:128 are never-read garbage)
    d_hsl = [nc.dram_tensor(f"hsl{l}", [NPC, 128], BF16) for l in range(2)]
    d_hsl.append(nc.dram_tensor("hsl2", [NPC, C], F32))
    d_hall = [nc.dram_tensor(f"hall{l}", [N, 128], BF16, addr_space="Shared")
              for l in range(2)]
    d_hall.append(nc.dram_tensor("hall2", [N, C], F32, addr_space="Shared"))

    RG = [list(range(NCORES))]

    def blob_ap(dram, layout, name):
        off, shape = layout[name]
        n = shape[0] * shape[1]
        return dram.ap()[0:1, off:off + n].rearrange(
            "a (p m) -> (a p) m", p=shape[0])

    with tile.TileContext(nc) as tc:
      for _rep in range(reps):
        with (
            tc.tile_pool(name="const", bufs=1) as cp,
            tc.tile_pool(name="hgp", bufs=2) as hgp,
            tc.tile_pool(name="msgp", bufs=6) as msgp,
            tc.tile_pool(name="wrp", bufs=TC + 2) as wrp,
            tc.tile_pool(name="tp", bufs=6) as tpp,
            tc.tile_pool(name="hcp", bufs=6) as hcp,
            tc.tile_pool(name="smf", bufs=6) as smf,
            tc.tile_pool(name="zp", bufs=2, space="PSUM") as zp,
            tc.tile_pool(name="aggp", bufs=1, space="PSUM") as aggp,
            tc.tile_pool(name="smp", bufs=1, space="PSUM") as smp,
        ):
            def bload(dram, layout, name, dtype, tag, shape=None):
                off, tshape = layout[name]
                shape = shape or tshape
                t = cp.tile(list(shape), dtype, tag=tag)
                nc.sync.dma_start(
                    out=t[0:tshape[0], :], in_=blob_ap(dram, layout, name))
                return t

            ea_sb = bload(d_bb, bb_l, "eaT7", BF16, "ea")
            w1_sb = bload(d_bb, bb_l, "W1p", BF16, "w1")
            w2_sb = bload(d_bb, bb_l, "W2p", BF16, "w2")
            w3_sb = bload(d_bb, bb_l, "W3p", BF16, "w3")
            xs_sb = bload(d_bf, bf_l, "xsrc", F32, "xs")
            xT2_sb = bload(d_bf, bf_l, "xT2", F32, "xT2")
            invd_sb = bload(d_bf, bf_l, "invd", F32, "invd")
            r1_sb = bload(d_bf, bf_l, "r1", F32, "r1")
            r2_sb = bload(d_bf, bf_l, "r2", F32, "r2")
            r3_sb = bload(d_bf, bf_l, "r3", F32, "r3")
            dst_sb = bload(d_bf, bf_l, "dstloc", F32, "dst")
            csel_sb = bload(d_bf, bf_l, "colsel", F32, "csel")
            # gather idx: load wrapped [16, Tt*8] then replicate to 128 rows
            ix_sb = cp.tile([128, Tt * 8], I16, tag="ix")
            nc.sync.dma_start(out=ix_sb[0:16, :],
                              in_=blob_ap(d_bi, bi_l, "idx"))
            nc.sync.dma_start(out=ix_sb[16:32, :], in_=ix_sb[0:16, :])
            nc.sync.dma_start(out=ix_sb[32:64, :], in_=ix_sb[0:32, :])
            nc.sync.dma_start(out=ix_sb[64:128, :], in_=ix_sb[0:64, :])

            # --------- on-device constant tables ---------
            iota128 = cp.tile([128, 128], F32, tag="iota128")
            nc.gpsimd.iota(iota128[:], pattern=[[1, 128]], base=0,
                           channel_multiplier=0,
                           allow_small_or_imprecise_dtypes=True)
            iotasel = cp.tile([128, 252], F32, tag="iotasel")
            nc.gpsimd.iota(iotasel[:], pattern=[[1, 252]], base=0,
                           channel_multiplier=0,
                           allow_small_or_imprecise_dtypes=True)
            idjp = cp.tile([128, 128], F32, tag="idjp")
            nc.gpsimd.iota(idjp[:], pattern=[[1, 128]], base=0,
                           channel_multiplier=-1,
                           allow_small_or_imprecise_dtypes=True)
            id32_sb = cp.tile([128, 128], F32, tag="id32")
            nc.vector.tensor_scalar(id32_sb[:], idjp[:], 0.0, None,
                                    Op.is_equal)
            sel_sb = cp.tile([128, 252], BF16, tag="sel")
            nc.vector.tensor_scalar(sel_sb[:], iotasel[:],
                                    csel_sb[:, 0:1], None, Op.is_equal)
            seln_sb = cp.tile([128, 128], F32, tag="seln")
            nc.vector.memset(seln_sb[0:32, :], -0.5)
            for qp in range(1, 4):
                nc.vector.memset(seln_sb[32 * qp:32 * (qp + 1), :], 0.0)
            # scatter one-hot S[e, n] = (dstloc[e,t] == n), built per tile
            S_sb = cp.tile([128, Ep], BF16, tag="S")
            for t in range(Tt):
                nc.vector.tensor_scalar(
                    S_sb[:, t * 128:(t + 1) * 128], iota128[:],
                    dst_sb[:, t:t + 1], None, Op.is_equal)

            # deterministic (never-read) padding columns of the gather tables
            zpad = cp.tile([128, 128 - C], BF16, tag="zpad")
            nc.vector.memset(zpad[:], 0.0)
            for l in range(2):
                for ch in range(2):
                    nc.sync.dma_start(
                        out=d_hsl[l][ch * 128:(ch + 1) * 128, C:128],
                        in_=zpad[:])

            hT1 = cp.tile([C + 1, NPC], F32, tag="hT1")
            hT2 = cp.tile([C + 1, NPC], F32, tag="hT2")
            hT3 = cp.tile([C, NPC], F32, tag="hT3")
            Rloc = cp.tile([128, 2], F32, tag="Rloc")
            nc.vector.memset(hT1[C:C + 1, :], 1.0)
            nc.vector.memset(hT2[C:C + 1, :], 1.0)

            # ---------------- layer 1 (in_c = 1) ----------------
            for ch in range(2):
                agg = aggp.tile([128, C], F32, tag="aggw")
                for t in range(TC):
                    gt = ch * TC + t
                    z1 = zp.tile([128, C], F32, tag="z")
                    nc.tensor.matmul(z1[:], ea_sb[:, gt * 128:(gt + 1) * 128],
                                     w1_sb[:], start=True, stop=True)
                    msg = msgp.tile([128, C], BF16, tag="msg")
                    nc.vector.tensor_scalar(
                        msg[:], z1[:], 0.0, xs_sb[:, gt:gt + 1],
                        Op.max, Op.mult)
                    nc.tensor.matmul(agg[:], S_sb[:, gt * 128:(gt + 1) * 128],
                                     msg[:], start=(t == 0), stop=(t == TC - 1))
                rtp = smp.tile([128, C], F32, tag="root")
                nc.tensor.matmul(rtp[:], xT2_sb[:, ch * 128:(ch + 1) * 128],
                                 r1_sb[:], start=True, stop=True)
                sm = smf.tile([128, C], F32, tag="sm")
                nc.vector.tensor_scalar(sm[:], agg[:],
                                        invd_sb[:, ch:ch + 1], None, Op.mult)
                hf_c = hcp.tile([128, C], F32, tag="hf")
                nc.vector.tensor_tensor(hf_c[:], sm[:], rtp[:], Op.add)
                nc.vector.tensor_scalar(hf_c[:], hf_c[:], 0.0, None, Op.max)
                h_c = hcp.tile([128, C], BF16, tag="hc")
                nc.scalar.copy(h_c[:], hf_c[:])
                tp = smp.tile([32, 128], F32, tag="tp")
                nc.tensor.transpose(tp[:], hf_c[:], id32_sb[:])
                nc.scalar.copy(hT1[0:C, ch * 128:(ch + 1) * 128], tp[:])
                nc.sync.dma_start(out=d_hsl[0][ch * 128:(ch + 1) * 128, 0:C],
                                  in_=h_c[:])
            def allgather(l):
                if "cc" in ablate:
                    nc.sync.dma_start(out=d_hall[l][0:NPC, :],
                                      in_=d_hsl[l].ap())
                else:
                    nc.gpsimd.collective_compute(
                        "AllGather", Op.bypass, replica_groups=RG,
                        ins=[d_hsl[l].ap()], outs=[d_hall[l].ap()])

            def phase_a(w_sb):
                """h-independent edge-MLP for chunk 0 of the next layer;
                emitted before the AllGather so PE/ACT overlap it."""
                tiles = []
                for t in range(TC):
                    z = zp.tile([128, C * C], F32, tag="z")
                    for q in range(2):
                        nc.tensor.matmul(
                            z[:, q * 512:(q + 1) * 512],
                            ea_sb[:, t * 128:(t + 1) * 128],
                            w_sb[:, q * 512:(q + 1) * 512],
                            start=True, stop=True)
                    wr = wrp.tile([128, C * C], BF16, tag="wr")
                    if t % 4 == 1:
                        nc.vector.tensor_scalar(wr[:], z[:], 0.0, None, Op.max)
                    else:
                        nc.scalar.activation(
                            wr[:], z[:], mybir.ActivationFunctionType.Relu)
                    tiles.append(wr)
                return tiles

            wrA = phase_a(w2_sb)
            allgather(0)

            # ---------------- layers 2 and 3 ----------------
            for li, (w_sb, r_sb, hTprev, hTcur) in enumerate(
                    [(w2_sb, r2_sb, hT1, hT2), (w3_sb, r3_sb, hT2, hT3)]):
                tab = d_hall[li]
                for ch in range(2):
                    hg = hgp.tile([128, TC, 128], BF16, tag="hg")
                    if "gather" in ablate:
                        # timing ablation: same bytes via plain DMA
                        done = 0
                        while done < TC:
                            nt = min(16, TC - done)
                            nc.sync.dma_start(
                                out=hg[:, done:done + nt, :],
                                in_=tab.ap()[0:nt * 128, :].rearrange(
                                    "(t p) e -> p t e", p=128))
                            done += nt
                    else:
                        # <=512 indices per dma_gather: one 4224-idx gather
                        # overflows the SWDGE descriptor ring (hang).
                        for g in range((TC * 128 + 511) // 512):
                            n_idx = min(512, TC * 128 - g * 512)
                            base = ch * TC * 8 + g * 32
                            nc.gpsimd.dma_gather(
                                out_ap=hg[:, g * 4:g * 4 + (n_idx + 127) // 128, :],
                                in_ap=tab.ap(),
                                idxs_ap=ix_sb[:, base:base + (n_idx + 15) // 16],
                                num_idxs=n_idx, num_idxs_reg=n_idx,
                                elem_size=128)
                    aggw = aggp.tile([128, C * C], F32, tag="aggw")
                    for t in range(TC):
                        gt = ch * TC + t
                        if ch == 0:
                            wr = wrA[t]
                        else:
                            z = zp.tile([128, C * C], F32, tag="z")
                            for q in range(2):
                                nc.tensor.matmul(
                                    z[:, q * 512:(q + 1) * 512],
                                    ea_sb[:, gt * 128:(gt + 1) * 128],
                                    w_sb[:, q * 512:(q + 1) * 512],
                                    start=True, stop=True)
                            wr = wrp.tile([128, C * C], BF16, tag="wr")
                            if t % 4 == 1:
                                nc.vector.tensor_scalar(
                                    wr[:], z[:], 0.0, None, Op.max)
                            else:
                                nc.scalar.activation(
                                    wr[:], z[:],
                                    mybir.ActivationFunctionType.Relu)
                        tt = tpp.tile([128, C * C], BF16, tag="t")
                        t3 = tt[:].rearrange("p (o i) -> p o i", i=C)
                        hgb = hg[:, t:t + 1, 0:C].broadcast_to([128, C, C])
                        wr3 = wr[:].rearrange("p (o i) -> p o i", i=C)
                        nc.vector.tensor_tensor(t3, wr3, hgb, Op.mult)
                        for q in range(2):
                            nc.tensor.matmul(
                                aggw[:, q * 512:(q + 1) * 512],
                                S_sb[:, gt * 128:(gt + 1) * 128],
                                tt[:, q * 512:(q + 1) * 512],
                                start=(t == 0), stop=(t == TC - 1))
                    rtp = smp.tile([128, C], F32, tag="root")
                    nc.tensor.matmul(rtp[:],
                                     hTprev[:, ch * 128:(ch + 1) * 128],
                                     r_sb[:], start=True, stop=True)
                    red = smf.tile([128, C], F32, tag="red")
                    nc.vector.tensor_reduce(
                        red[:], aggw[:].rearrange("p (o i) -> p o i", i=C),
                        mybir.AxisListType.X, Op.add)
                    sm = smf.tile([128, C], F32, tag="sm")
                    nc.vector.tensor_scalar(sm[:], red[:],
                                            invd_sb[:, ch:ch + 1], None,
                                            Op.mult)
                    hf_c = hcp.tile([128, C], F32, tag="hf")
                    nc.vector.tensor_tensor(hf_c[:], sm[:], rtp[:], Op.add)
                    nc.vector.tensor_scalar(hf_c[:], hf_c[:], 0.0, None,
                                            Op.max)
                    if li == 1:
                        nc.vector.tensor_reduce(
                            Rloc[:, ch:ch + 1], hf_c[:],
                            mybir.AxisListType.X, Op.add)
                    tp = smp.tile([32, 128], F32, tag="tp")
                    nc.tensor.transpose(tp[:], hf_c[:], id32_sb[:])
                    nc.scalar.copy(hTcur[0:C, ch * 128:(ch + 1) * 128], tp[:])
                    if li == 0:
                        h_c = hcp.tile([128, C], BF16, tag="hc")
                        nc.scalar.copy(h_c[:], hf_c[:])
                        nc.sync.dma_start(
                            out=d_hsl[1][ch * 128:(ch + 1) * 128, 0:C],
                            in_=h_c[:])
                    else:
                        nc.sync.dma_start(
                            out=d_hsl[2][ch * 128:(ch + 1) * 128, :],
                            in_=hf_c[:])
                if li == 0:
                    wrA = phase_a(w3_sb)
                allgather(li + 1)

            # ---------------- CBT prep ----------------
            # f32 differencing: bf16 h here blows up per-entry relative
            # error on near-zero CBT entries (cancellation), keep f32
            hTrep = cp.tile([128, N], F32, tag="hTrep")
            h3f = cp.tile([128, 16, C], F32, tag="h3f")
            nc.sync.dma_start(
                out=h3f[:], in_=d_hall[2].ap().rearrange("(a b) c -> b a c",
                                                         a=16))
            for j in range(16):
                tp = smp.tile([32, 128], F32, tag="tp" if j % 2 else "root")
                nc.tensor.transpose(tp[:], h3f[:, j, :], id32_sb[:])
                nc.scalar.copy(hTrep[0:32, j * 128:(j + 1) * 128], tp[:])
            nc.sync.dma_start(out=hTrep[32:64, :], in_=hTrep[0:32, :])
            nc.sync.dma_start(out=hTrep[64:128, :], in_=hTrep[0:64, :])
            htcols = cp.tile([128, NPC // 4], F32, tag="htcols")
            htcolsn = cp.tile([128, NPC // 4], F32, tag="htcolsn")
            for s in range(4):
                nc.sync.dma_start(
                    out=htcols[s * 32:(s + 1) * 32, :],
                    in_=hT3[0:C, s::4])
            nc.vector.tensor_scalar(htcolsn[:], htcols[:], -1.0, None,
                                    Op.mult)

        # ---------------- CBT ----------------
        with (
            tc.tile_pool(name="ap_", bufs=5) as apool,
            tc.tile_pool(name="ob", bufs=4) as ob,
            tc.tile_pool(name="cbtp", bufs=2, space="PSUM") as cbtp,
        ):
            for ib in range(2):
                cps = cbtp.tile([128, N], F32, tag="cps")
                for b in range(32):
                    col = ib * 32 + b
                    a_b = apool.tile([128, N], BF16, tag="a")
                    if b % 7 >= 4:
                        nc.scalar.activation(
                            a_b[:], hTrep[:],
                            mybir.ActivationFunctionType.Relu,
                            bias=htcolsn[:, col:col + 1])
                    else:
                        nc.vector.tensor_scalar(
                            a_b[:], hTrep[:], htcols[:, col:col + 1], 0.0,
                            Op.subtract, Op.max)
                    lo = 124 - 4 * b
                    for q in range(4):
                        nc.tensor.matmul(
                            cps[:, q * 512:(q + 1) * 512],
                            sel_sb[:, lo:lo + 128],
                            a_b[:, q * 512:(q + 1) * 512],
                            start=(b == 0), stop=False)
                for q in range(4):
                    nc.tensor.matmul(
                        cps[:, q * 512:(q + 1) * 512], seln_sb[:],
                        hTrep[:, q * 512:(q + 1) * 512],
                        start=False, stop=True)
                for q in range(4):
                    ob_t = ob.tile([128, 512], BF16, tag="ob")
                    if q % 2 == 0:
                        nc.scalar.activation(
                            ob_t[:], cps[:, q * 512:(q + 1) * 512],
                            mybir.ActivationFunctionType.Identity,
                            bias=Rloc[:, ib:ib + 1], scale=2.0)
                    else:
                        nc.vector.tensor_scalar(
                            ob_t[:], cps[:, q * 512:(q + 1) * 512],
                            2.0, Rloc[:, ib:ib + 1], Op.mult, Op.add)
                    nc.sync.dma_start(
                        out=d_out[ib * 128:(ib + 1) * 128,
                                  q * 512:(q + 1) * 512],
                        in_=ob_t[:])

    nc.compile()
    return nc


# --------------------------------------------------------------------------
# runner: custom PJRT dispatch (no zero-output upload, 3 input arrays)
# --------------------------------------------------------------------------

class _Runner:
    def __init__(self, TC, reps=1):
        import jax
        from jax.sharding import Mesh, PartitionSpec
        from jax.experimental.shard_map import shard_map
        from concourse import bass2jax
        bass2jax.install_neuronx_cc_hook()

        self.jax = jax
        nc = build_program(TC, reps=reps)
        self.nc = nc
        partition_name = (nc.partition_id_tensor.name
                          if nc.partition_id_tensor else None)
        in_names, out_names, out_avals = [], [], []
        for alloc in nc.m.functions[0].allocations:
            if not isinstance(alloc, mybir.MemoryLocationSet):
                continue
            name = alloc.memorylocations[0].name
            if alloc.kind == "ExternalInput":
                if name != partition_name:
                    in_names.append(name)
            elif alloc.kind == "ExternalOutput":
                out_names.append(name)
                out_avals.append(jax.core.ShapedArray(
                    tuple(alloc.tensor_shape), mybir.dt.np(alloc.dtype)))
        self.in_names = in_names
        self.out_names = out_names
        all_in = list(in_names)
        if partition_name is not None:
            all_in.append(partition_name)

        def _body(*args):
            operands = list(args)
            if partition_name is not None:
                operands.append(bass2jax.partition_id_tensor())
            outs = bass2jax._bass_exec_p.bind(
                *operands, out_avals=tuple(out_avals),
                in_names=tuple(all_in), out_names=tuple(out_names),
                lowering_input_output_aliases=(),
                sim_require_finite=True, sim_require_nnan=True, nc=nc)
            return tuple(outs)

        devices = jax.devices()[:NCORES]
        mesh = Mesh(np.asarray(devices), ("core",))
        self.sharded = jax.jit(shard_map(
            _body, mesh=mesh,
            in_specs=(PartitionSpec("core"),) * len(in_names),
            out_specs=(PartitionSpec("core"),) * len(out_names),
            check_rep=False), keep_unused=True)

    def global_args(self, in_maps):
        return [np.concatenate([in_maps[c][nm] for c in range(NCORES)], 0)
                for nm in self.in_names]

    def __call__(self, in_maps):
        out = self.sharded(*self.global_args(in_maps))
        self.jax.block_until_ready(out)
        return np.asarray(out[0])


def _get_runner(TC):
    r = _RUNNER_CACHE.get(TC)
    if r is None:
        r = _Runner(TC)
        _RUNNER_CACHE[TC] = r
    return r


def kernel(**inputs):
    TC, in_maps = _prep(**inputs)
    runner = _get_runner(TC)
    out = runner(in_maps)                    # [NCORES*NPC, N] bf16
    return np.asarray(out, dtype=np.float32)


# revision 36
# speedup vs baseline: 1.0615x; 1.0147x over previous
"""Trainium2 Bass kernel for nn_DGN (3x NNConv GNN + all-pairs L1 CBT).

Strategy (8 NeuronCores, SPMD):
  - Edges sorted by (dst, src), sharded so core c owns destination nodes
    [256c, 256(c+1)) as two 128-node chunks; per-(core,chunk) edge lists are
    padded to a uniform tile count so a single SPMD program serves all cores.
  - Per 128-edge tile: PE computes the edge-MLP z = ea' @ W' (bias folded via
    a ones-row), DVE/ACT fuse relu+multiply-by-gathered-source-features, and
    PE scatter-matmuls (one-hot lhsT) accumulate the per-node mean and the
    i-contraction in a wide [n, out*in] PSUM accumulator, reduced at chunk
    end.  Root-weight term is injected as one extra matmul into its own PSUM.
  - Source-feature gather runs as a SWDGE dma_gather straight from the
    AllGather output: h slices are stored into 128-wide (256B-aligned) rows
    so the collective result doubles as the gather table, and the next
    layer's h-independent edge-MLP tiles (z matmul + relu) are emitted
    before each collective so PE/ACT hide its ~50us latency.
  - CBT: per 4-row batch of local output rows, one DVE tensor_scalar
    (subtract, max 0) against a 4x-replicated transposed-h tile produces
    relu(h[j,k]-h[i,k]) for all j,k; a selection matmul on PE sums over k
    into a [128, 2048] PSUM block accumulated over 32 batches, using
    sum|d| = 2*sum relu(d) - (R_j - R_i).

Perf notes (axon tunnel, 2026-08-08): per-exec wall time is dominated by a
~75 ms PJRT-over-axon dispatch floor plus input upload (~6 ms per array +
~160 MB/s).  So all large constant tables (the one-hot scatter matrix S,
selection/identity matrices, replicated gather indices) are built on-device
from a few KB of packed indices, inputs ship as three per-dtype blobs, the
zero output buffers are never uploaded (the custom call allocates outputs
fresh; every element of d_out is written), and the output is bf16.
"""

import numpy as np
import ml_dtypes

import concourse.bass as bass
import concourse.bacc as bacc
import concourse.tile as tile
import concourse.mybir as mybir

BF16 = mybir.dt.bfloat16
F32 = mybir.dt.float32
I16 = mybir.dt.int16

N = 2048
E = 65536
NV = 6
C = 32
NCORES = 8
NPC = N // NCORES      # nodes per core = 256
CHUNK = 128            # node chunk (PSUM partition dim)
Op = mybir.AluOpType

_RUNNER_CACHE = {}


# --------------------------------------------------------------------------
# host-side prep
# --------------------------------------------------------------------------

def _bf16(a):
    return np.asarray(a, dtype=np.float32).astype(ml_dtypes.bfloat16)


def _permute_w(Wf, b):
    """[NV, in*out] + [in*out] -> [7, in*out] with columns re-ordered from
    (i-major) i*out+o to (o-major) o*in+i, bias folded as last row."""
    in_c = Wf.shape[1] // C
    Wb = np.concatenate([Wf, b[None, :]], axis=0)  # [7, in*out]
    cols = Wb.reshape(NV + 1, in_c, C)             # [7, i, o]
    return np.transpose(cols, (0, 2, 1)).reshape(NV + 1, in_c * C)  # (o, i)


def _layout(TC):
    """Byte offsets of every tensor inside the three per-dtype blobs."""
    Tt = 2 * TC
    Ep = Tt * 128
    bb = dict()  # bf16 blob: name -> (offset_elems, shape)
    off = 0
    for name, shape in [("eaT7", (7, Ep)), ("W1p", (7, C)),
                        ("W2p", (7, C * C)), ("W3p", (7, C * C)),
                        ("idx", (16, Tt * 8))]:  # i16 bits carried as bf16
        bb[name] = (off, shape)
        off += shape[0] * shape[1]
    nb_b = off
    bf = dict()  # f32 blob
    off = 0
    for name, shape in [("xsrc", (128, Tt)), ("xT2", (2, NPC)),
                        ("invd", (128, 2)), ("r1", (2, C)),
                        ("r2", (C + 1, C)), ("r3", (C + 1, C)),
                        ("dstloc", (128, Tt)), ("colsel", (128, 1))]:
        bf[name] = (off, shape)
        off += shape[0] * shape[1]
    nb_f = off
    return (bb, nb_b), (bf, nb_f)


def _prep(x, edge_attr, edge_index, W1, b1, root1, bias1, W2, b2, root2,
          bias2, W3, b3, root3, bias3):
    src = np.asarray(edge_index[0], dtype=np.int64)
    dst = np.asarray(edge_index[1], dtype=np.int64)
    x = np.asarray(x, dtype=np.float32)
    ea = np.asarray(edge_attr, dtype=np.float32)

    deg = np.bincount(dst, minlength=N).astype(np.float64)
    inv_deg = (1.0 / np.maximum(deg, 1.0)).astype(np.float32)

    order = np.lexsort((src, dst))
    s_srt, d_srt = src[order], dst[order]

    # per 128-dst-chunk edge index lists (into the original edge arrays)
    groups = []
    for g in range(N // CHUNK):          # 16 chunks
        sel = order[(d_srt >= g * CHUNK) & (d_srt < (g + 1) * CHUNK)]
        groups.append(sel)
    TC = max(1, max((len(g) + 127) // 128 for g in groups))
    Tt = 2 * TC
    Ep = Tt * 128

    (bb_l, nb_b), (bf_l, nb_f) = _layout(TC)

    shared_b = {
        "W1p": _bf16(np.concatenate([W1, b1[None, :]], 0)),
        "W2p": _bf16(_permute_w(W2, b2)),
        "W3p": _bf16(_permute_w(W3, b3)),
    }
    shared_f = {
        "r1": np.stack([root1[0], bias1], 0).astype(np.float32),
        "r2": np.concatenate([root2, bias2[None, :]], 0).astype(np.float32),
        "r3": np.concatenate([root3, bias3[None, :]], 0).astype(np.float32),
        "colsel": (124 + np.arange(128) // 32).astype(np.float32)[:, None],
    }

    in_maps = []
    for c in range(NCORES):
        eaT7 = np.zeros((7, Ep), np.float32)
        xsrc = np.zeros((128, Tt), np.float32)
        dstloc = np.full((128, Tt), 300.0, np.float32)  # 300 = no-match pad
        idx16 = np.zeros((16, Tt * 8), np.int16)
        for ch in range(2):
            g = groups[2 * c + ch]
            n = len(g)
            base = ch * TC * 128
            eaT7[:NV, base:base + n] = ea[g].T
            eaT7[NV, base:base + n] = 1.0
            gs = src[g]
            gd = dst[g]
            # edge-slot (p, t) layout: slot j of chunk ch -> p=j%128, t=j//128
            p = np.arange(n) % 128
            t = ch * TC + np.arange(n) // 128
            xsrc[p, t] = x[gs, 0]
            dstloc[p, t] = (gd - (2 * c + ch) * CHUNK).astype(np.float32)
            # gather indices, wrapped: idx j -> [j%16, j//16]
            ids = np.zeros(TC * 128, np.int16)
            ids[:n] = gs.astype(np.int16)
            idx16[:, ch * TC * 8:(ch + 1) * TC * 8] = \
                ids.reshape(TC * 8, 16).T
        xT2 = np.zeros((2, NPC), np.float32)
        xT2[0] = x[c * NPC:(c + 1) * NPC, 0]
        xT2[1] = 1.0
        invd = inv_deg[c * NPC:(c + 1) * NPC].reshape(2, 128).T.copy()

        per_b = {"eaT7": _bf16(eaT7), **shared_b,
                 "idx": np.ascontiguousarray(idx16).view(ml_dtypes.bfloat16)}
        per_f = {"xsrc": xsrc, "xT2": xT2, "invd": invd,
                 "dstloc": dstloc, **shared_f}
        blob_b = np.zeros((1, nb_b), ml_dtypes.bfloat16)
        for name, (off, shape) in bb_l.items():
            blob_b[0, off:off + shape[0] * shape[1]] = \
                np.ascontiguousarray(per_b[name]).reshape(-1)
        blob_f = np.zeros((1, nb_f), np.float32)
        for name, (off, shape) in bf_l.items():
            blob_f[0, off:off + shape[0] * shape[1]] = \
                np.ascontiguousarray(per_f[name]).reshape(-1)
        in_maps.append({"bb": blob_b, "bf": blob_f})
    return TC, in_maps


# --------------------------------------------------------------------------
# device program
# --------------------------------------------------------------------------

def build_program(TC, reps=1, ablate=()):
    ablate = set(ablate)
    Tt = 2 * TC
    Ep = Tt * 128
    (bb_l, nb_b), (bf_l, nb_f) = _layout(TC)
    nc = bacc.Bacc("TRN2", target_bir_lowering=False, debug=False,
                   num_devices=NCORES)

    d_bb = nc.dram_tensor("bb", [1, nb_b], BF16, kind="ExternalInput")
    d_bf = nc.dram_tensor("bf", [1, nb_f], F32, kind="ExternalInput")
    d_out = nc.dram_tensor("out", [NPC, N], BF16, kind="ExternalOutput")

    # layers 0/1: 128-wide rows so the AllGather output doubles as the
    # 256B-aligned dma_gather table (cols C:128 are never-read garbage)
    d_hsl = [nc.dram_tensor(f"hsl{l}", [NPC, 128], BF16) for l in range(2)]
    d_hsl.append(nc.dram_tensor("hsl2", [NPC, C], F32))
    d_hall = [nc.dram_tensor(f"hall{l}", [N, 128], BF16, addr_space="Shared")
              for l in range(2)]
    d_hall.append(nc.dram_tensor("hall2", [N, C], F32, addr_space="Shared"))

    RG = [list(range(NCORES))]

    def blob_ap(dram, layout, name):
        off, shape = layout[name]
        n = shape[0] * shape[1]
        return dram.ap()[0:1, off:off + n].rearrange(
            "a (p m) -> (a p) m", p=shape[0])

    with tile.TileContext(nc) as tc:
      for _rep in range(reps):
        with (
            tc.tile_pool(name="const", bufs=1) as cp,
            tc.tile_pool(name="hgp", bufs=2) as hgp,
            tc.tile_pool(name="msgp", bufs=6) as msgp,
            tc.tile_pool(name="wrp", bufs=TC + 2) as wrp,
            tc.tile_pool(name="tp", bufs=6) as tpp,
            tc.tile_pool(name="hcp", bufs=6) as hcp,
            tc.tile_pool(name="smf", bufs=6) as smf,
            tc.tile_pool(name="zp", bufs=2, space="PSUM") as zp,
            tc.tile_pool(name="aggp", bufs=1, space="PSUM") as aggp,
            tc.tile_pool(name="smp", bufs=1, space="PSUM") as smp,
        ):
            def bload(dram, layout, name, dtype, tag, shape=None):
                off, tshape = layout[name]
                shape = shape or tshape
                t = cp.tile(list(shape), dtype, tag=tag)
                nc.sync.dma_start(
                    out=t[0:tshape[0], :], in_=blob_ap(dram, layout, name))
                return t

            ea_sb = bload(d_bb, bb_l, "eaT7", BF16, "ea")
            w1_sb = bload(d_bb, bb_l, "W1p", BF16, "w1")
            w2_sb = bload(d_bb, bb_l, "W2p", BF16, "w2")
            w3_sb = bload(d_bb, bb_l, "W3p", BF16, "w3")
            xs_sb = bload(d_bf, bf_l, "xsrc", F32, "xs")
            xT2_sb = bload(d_bf, bf_l, "xT2", F32, "xT2")
            invd_sb = bload(d_bf, bf_l, "invd", F32, "invd")
            r1_sb = bload(d_bf, bf_l, "r1", F32, "r1")
            r2_sb = bload(d_bf, bf_l, "r2", F32, "r2")
            r3_sb = bload(d_bf, bf_l, "r3", F32, "r3")
            dst_sb = bload(d_bf, bf_l, "dstloc", F32, "dst")
            csel_sb = bload(d_bf, bf_l, "colsel", F32, "csel")
            # gather idx: i16 bits ride the bf16 blob; load wrapped
            # [16, Tt*8] then replicate to 128 rows (bitcast at use site)
            ix_sb = cp.tile([128, Tt * 8], BF16, tag="ix")
            nc.sync.dma_start(out=ix_sb[0:16, :],
                              in_=blob_ap(d_bb, bb_l, "idx"))
            nc.sync.dma_start(out=ix_sb[16:32, :], in_=ix_sb[0:16, :])
            nc.sync.dma_start(out=ix_sb[32:64, :], in_=ix_sb[0:32, :])
            nc.sync.dma_start(out=ix_sb[64:128, :], in_=ix_sb[0:64, :])

            # --------- on-device constant tables ---------
            iota128 = cp.tile([128, 128], F32, tag="iota128")
            nc.gpsimd.iota(iota128[:], pattern=[[1, 128]], base=0,
                           channel_multiplier=0,
                           allow_small_or_imprecise_dtypes=True)
            iotasel = cp.tile([128, 252], F32, tag="iotasel")
            nc.gpsimd.iota(iotasel[:], pattern=[[1, 252]], base=0,
                           channel_multiplier=0,
                           allow_small_or_imprecise_dtypes=True)
            idjp = cp.tile([128, 128], F32, tag="idjp")
            nc.gpsimd.iota(idjp[:], pattern=[[1, 128]], base=0,
                           channel_multiplier=-1,
                           allow_small_or_imprecise_dtypes=True)
            id32_sb = cp.tile([128, 128], F32, tag="id32")
            nc.vector.tensor_scalar(id32_sb[:], idjp[:], 0.0, None,
                                    Op.is_equal)
            sel_sb = cp.tile([128, 252], BF16, tag="sel")
            nc.vector.tensor_scalar(sel_sb[:], iotasel[:],
                                    csel_sb[:, 0:1], None, Op.is_equal)
            seln_sb = cp.tile([128, 128], F32, tag="seln")
            nc.vector.memset(seln_sb[0:32, :], -0.5)
            for qp in range(1, 4):
                nc.vector.memset(seln_sb[32 * qp:32 * (qp + 1), :], 0.0)
            # scatter one-hot S[e, n] = (dstloc[e,t] == n), built per tile
            S_sb = cp.tile([128, Ep], BF16, tag="S")
            for t in range(Tt):
                nc.vector.tensor_scalar(
                    S_sb[:, t * 128:(t + 1) * 128], iota128[:],
                    dst_sb[:, t:t + 1], None, Op.is_equal)

            # deterministic (never-read) padding columns of the gather tables
            zpad = cp.tile([128, 128 - C], BF16, tag="zpad")
            nc.vector.memset(zpad[:], 0.0)
            for l in range(2):
                for ch in range(2):
                    nc.sync.dma_start(
                        out=d_hsl[l][ch * 128:(ch + 1) * 128, C:128],
                        in_=zpad[:])

            hT1 = cp.tile([C + 1, NPC], F32, tag="hT1")
            hT2 = cp.tile([C + 1, NPC], F32, tag="hT2")
            hT3 = cp.tile([C, NPC], F32, tag="hT3")
            Rloc = cp.tile([128, 2], F32, tag="Rloc")
            nc.vector.memset(hT1[C:C + 1, :], 1.0)
            nc.vector.memset(hT2[C:C + 1, :], 1.0)

            # ---------------- layer 1 (in_c = 1) ----------------
            for ch in range(2):
                agg = aggp.tile([128, C], F32, tag="aggw")
                for t in range(TC):
                    gt = ch * TC + t
                    z1 = zp.tile([128, C], F32, tag="z")
                    nc.tensor.matmul(z1[:], ea_sb[:, gt * 128:(gt + 1) * 128],
                                     w1_sb[:], start=True, stop=True)
                    msg = msgp.tile([128, C], BF16, tag="msg")
                    nc.vector.tensor_scalar(
                        msg[:], z1[:], 0.0, xs_sb[:, gt:gt + 1],
                        Op.max, Op.mult)
                    nc.tensor.matmul(agg[:], S_sb[:, gt * 128:(gt + 1) * 128],
                                     msg[:], start=(t == 0), stop=(t == TC - 1))
                rtp = smp.tile([128, C], F32, tag="root")
                nc.tensor.matmul(rtp[:], xT2_sb[:, ch * 128:(ch + 1) * 128],
                                 r1_sb[:], start=True, stop=True)
                sm = smf.tile([128, C], F32, tag="sm")
                nc.vector.tensor_scalar(sm[:], agg[:],
                                        invd_sb[:, ch:ch + 1], None, Op.mult)
                hf_c = hcp.tile([128, C], F32, tag="hf")
                nc.vector.tensor_tensor(hf_c[:], sm[:], rtp[:], Op.add)
                nc.vector.tensor_scalar(hf_c[:], hf_c[:], 0.0, None, Op.max)
                h_c = hcp.tile([128, C], BF16, tag="hc")
                nc.scalar.copy(h_c[:], hf_c[:])
                tp = smp.tile([32, 128], F32, tag="tp")
                nc.tensor.transpose(tp[:], hf_c[:], id32_sb[:])
                nc.scalar.copy(hT1[0:C, ch * 128:(ch + 1) * 128], tp[:])
                nc.sync.dma_start(out=d_hsl[0][ch * 128:(ch + 1) * 128, 0:C],
                                  in_=h_c[:])
            def allgather(l):
                if "cc" in ablate:
                    nc.sync.dma_start(out=d_hall[l][0:NPC, :],
                                      in_=d_hsl[l].ap())
                else:
                    nc.gpsimd.collective_compute(
                        "AllGather", Op.bypass, replica_groups=RG,
                        ins=[d_hsl[l].ap()], outs=[d_hall[l].ap()])

            def phase_a(w_sb):
                """h-independent edge-MLP for chunk 0 of the next layer;
                emitted before the AllGather so PE/ACT overlap it."""
                tiles = []
                for t in range(TC):
                    z = zp.tile([128, C * C], F32, tag="z")
                    for q in range(2):
                        nc.tensor.matmul(
                            z[:, q * 512:(q + 1) * 512],
                            ea_sb[:, t * 128:(t + 1) * 128],
                            w_sb[:, q * 512:(q + 1) * 512],
                            start=True, stop=True)
                    wr = wrp.tile([128, C * C], BF16, tag="wr")
                    if t % 4 == 1:
                        nc.vector.tensor_scalar(wr[:], z[:], 0.0, None, Op.max)
                    else:
                        nc.scalar.activation(
                            wr[:], z[:], mybir.ActivationFunctionType.Relu)
                    tiles.append(wr)
                return tiles

            wrA = phase_a(w2_sb)
            allgather(0)

            # ---------------- layers 2 and 3 ----------------
            for li, (w_sb, r_sb, hTprev, hTcur) in enumerate(
                    [(w2_sb, r2_sb, hT1, hT2), (w3_sb, r3_sb, hT2, hT3)]):
                tab = d_hall[li]
                for ch in range(2):
                    hg = hgp.tile([128, TC, 128], BF16, tag="hg")
                    if "gather" in ablate:
                        # timing ablation: same bytes via plain DMA
                        done = 0
                        while done < TC:
                            nt = min(16, TC - done)
                            nc.sync.dma_start(
                                out=hg[:, done:done + nt, :],
                                in_=tab.ap()[0:nt * 128, :].rearrange(
                                    "(t p) e -> p t e", p=128))
                            done += nt
                    else:
                        # <=512 indices per dma_gather: one 4224-idx gather
                        # overflows the SWDGE descriptor ring (hang).
                        for g in range((TC * 128 + 511) // 512):
                            n_idx = min(512, TC * 128 - g * 512)
                            base = ch * TC * 8 + g * 32
                            nc.gpsimd.dma_gather(
                                out_ap=hg[:, g * 4:g * 4 + (n_idx + 127) // 128, :],
                                in_ap=tab.ap(),
                                idxs_ap=ix_sb[:, base:base + (n_idx + 15) // 16
                                              ].bitcast(I16),
                                num_idxs=n_idx, num_idxs_reg=n_idx,
                                elem_size=128)
                    aggw = aggp.tile([128, C * C], F32, tag="aggw")
                    for t in range(TC):
                        gt = ch * TC + t
                        if ch == 0:
                            wr = wrA[t]
                        else:
                            z = zp.tile([128, C * C], F32, tag="z")
                            for q in range(2):
                                nc.tensor.matmul(
                                    z[:, q * 512:(q + 1) * 512],
                                    ea_sb[:, gt * 128:(gt + 1) * 128],
                                    w_sb[:, q * 512:(q + 1) * 512],
                                    start=True, stop=True)
                            wr = wrp.tile([128, C * C], BF16, tag="wr")
                            if t % 4 == 1:
                                nc.vector.tensor_scalar(
                                    wr[:], z[:], 0.0, None, Op.max)
                            else:
                                nc.scalar.activation(
                                    wr[:], z[:],
                                    mybir.ActivationFunctionType.Relu)
                        tt = tpp.tile([128, C * C], BF16, tag="t")
                        t3 = tt[:].rearrange("p (o i) -> p o i", i=C)
                        hgb = hg[:, t:t + 1, 0:C].broadcast_to([128, C, C])
                        wr3 = wr[:].rearrange("p (o i) -> p o i", i=C)
                        nc.vector.tensor_tensor(t3, wr3, hgb, Op.mult)
                        for q in range(2):
                            nc.tensor.matmul(
                                aggw[:, q * 512:(q + 1) * 512],
                                S_sb[:, gt * 128:(gt + 1) * 128],
                                tt[:, q * 512:(q + 1) * 512],
                                start=(t == 0), stop=(t == TC - 1))
                    rtp = smp.tile([128, C], F32, tag="root")
                    nc.tensor.matmul(rtp[:],
                                     hTprev[:, ch * 128:(ch + 1) * 128],
                                     r_sb[:], start=True, stop=True)
                    red = smf.tile([128, C], F32, tag="red")
                    nc.vector.tensor_reduce(
                        red[:], aggw[:].rearrange("p (o i) -> p o i", i=C),
                        mybir.AxisListType.X, Op.add)
                    sm = smf.tile([128, C], F32, tag="sm")
                    nc.vector.tensor_scalar(sm[:], red[:],
                                            invd_sb[:, ch:ch + 1], None,
                                            Op.mult)
                    hf_c = hcp.tile([128, C], F32, tag="hf")
                    nc.vector.tensor_tensor(hf_c[:], sm[:], rtp[:], Op.add)
                    nc.vector.tensor_scalar(hf_c[:], hf_c[:], 0.0, None,
                                            Op.max)
                    if li == 1:
                        nc.vector.tensor_reduce(
                            Rloc[:, ch:ch + 1], hf_c[:],
                            mybir.AxisListType.X, Op.add)
                    tp = smp.tile([32, 128], F32, tag="tp")
                    nc.tensor.transpose(tp[:], hf_c[:], id32_sb[:])
                    nc.scalar.copy(hTcur[0:C, ch * 128:(ch + 1) * 128], tp[:])
                    if li == 0:
                        h_c = hcp.tile([128, C], BF16, tag="hc")
                        nc.scalar.copy(h_c[:], hf_c[:])
                        nc.sync.dma_start(
                            out=d_hsl[1][ch * 128:(ch + 1) * 128, 0:C],
                            in_=h_c[:])
                    else:
                        nc.sync.dma_start(
                            out=d_hsl[2][ch * 128:(ch + 1) * 128, :],
                            in_=hf_c[:])
                if li == 0:
                    wrA = phase_a(w3_sb)
                allgather(li + 1)

            # ---------------- CBT prep ----------------
            # f32 differencing: bf16 h here blows up per-entry relative
            # error on near-zero CBT entries (cancellation), keep f32
            hTrep = cp.tile([128, N], F32, tag="hTrep")
            h3f = cp.tile([128, 16, C], F32, tag="h3f")
            nc.sync.dma_start(
                out=h3f[:], in_=d_hall[2].ap().rearrange("(a b) c -> b a c",
                                                         a=16))
            for j in range(16):
                tp = smp.tile([32, 128], F32, tag="tp" if j % 2 else "root")
                nc.tensor.transpose(tp[:], h3f[:, j, :], id32_sb[:])
                nc.scalar.copy(hTrep[0:32, j * 128:(j + 1) * 128], tp[:])
            nc.sync.dma_start(out=hTrep[32:64, :], in_=hTrep[0:32, :])
            nc.sync.dma_start(out=hTrep[64:128, :], in_=hTrep[0:64, :])
            htcols = cp.tile([128, NPC // 4], F32, tag="htcols")
            htcolsn = cp.tile([128, NPC // 4], F32, tag="htcolsn")
            for s in range(4):
                nc.sync.dma_start(
                    out=htcols[s * 32:(s + 1) * 32, :],
                    in_=hT3[0:C, s::4])
            nc.vector.tensor_scalar(htcolsn[:], htcols[:], -1.0, None,
                                    Op.mult)

        # ---------------- CBT ----------------
        with (
            tc.tile_pool(name="ap_", bufs=5) as apool,
            tc.tile_pool(name="ob", bufs=4) as ob,
            tc.tile_pool(name="cbtp", bufs=2, space="PSUM") as cbtp,
        ):
            for ib in range(2):
                cps = cbtp.tile([128, N], F32, tag="cps")
                for b in range(32):
                    col = ib * 32 + b
                    a_b = apool.tile([128, N], BF16, tag="a")
                    if b % 7 >= 4:
                        nc.scalar.activation(
                            a_b[:], hTrep[:],
                            mybir.ActivationFunctionType.Relu,
                            bias=htcolsn[:, col:col + 1])
                    else:
                        nc.vector.tensor_scalar(
                            a_b[:], hTrep[:], htcols[:, col:col + 1], 0.0,
                            Op.subtract, Op.max)
                    lo = 124 - 4 * b
                    for q in range(4):
                        nc.tensor.matmul(
                            cps[:, q * 512:(q + 1) * 512],
                            sel_sb[:, lo:lo + 128],
                            a_b[:, q * 512:(q + 1) * 512],
                            start=(b == 0), stop=False)
                for q in range(4):
                    nc.tensor.matmul(
                        cps[:, q * 512:(q + 1) * 512], seln_sb[:],
                        hTrep[:, q * 512:(q + 1) * 512],
                        start=False, stop=True)
                for q in range(4):
                    ob_t = ob.tile([128, 512], BF16, tag="ob")
                    if q % 2 == 0:
                        nc.scalar.activation(
                            ob_t[:], cps[:, q * 512:(q + 1) * 512],
                            mybir.ActivationFunctionType.Identity,
                            bias=Rloc[:, ib:ib + 1], scale=2.0)
                    else:
                        nc.vector.tensor_scalar(
                            ob_t[:], cps[:, q * 512:(q + 1) * 512],
                            2.0, Rloc[:, ib:ib + 1], Op.mult, Op.add)
                    nc.sync.dma_start(
                        out=d_out[ib * 128:(ib + 1) * 128,
                                  q * 512:(q + 1) * 512],
                        in_=ob_t[:])

    nc.compile()
    return nc


# --------------------------------------------------------------------------
# runner: custom PJRT dispatch (no zero-output upload, 3 input arrays)
# --------------------------------------------------------------------------

class _Runner:
    def __init__(self, TC, reps=1):
        import jax
        from jax.sharding import Mesh, PartitionSpec
        from jax.experimental.shard_map import shard_map
        from concourse import bass2jax
        bass2jax.install_neuronx_cc_hook()

        self.jax = jax
        nc = build_program(TC, reps=reps)
        self.nc = nc
        partition_name = (nc.partition_id_tensor.name
                          if nc.partition_id_tensor else None)
        in_names, out_names, out_avals = [], [], []
        for alloc in nc.m.functions[0].allocations:
            if not isinstance(alloc, mybir.MemoryLocationSet):
                continue
            name = alloc.memorylocations[0].name
            if alloc.kind == "ExternalInput":
                if name != partition_name:
                    in_names.append(name)
            elif alloc.kind == "ExternalOutput":
                out_names.append(name)
                out_avals.append(jax.core.ShapedArray(
                    tuple(alloc.tensor_shape), mybir.dt.np(alloc.dtype)))
        self.in_names = in_names
        self.out_names = out_names
        all_in = list(in_names)
        if partition_name is not None:
            all_in.append(partition_name)

        def _body(*args):
            operands = list(args)
            if partition_name is not None:
                operands.append(bass2jax.partition_id_tensor())
            outs = bass2jax._bass_exec_p.bind(
                *operands, out_avals=tuple(out_avals),
                in_names=tuple(all_in), out_names=tuple(out_names),
                lowering_input_output_aliases=(),
                sim_require_finite=True, sim_require_nnan=True, nc=nc)
            return tuple(outs)

        devices = jax.devices()[:NCORES]
        mesh = Mesh(np.asarray(devices), ("core",))
        self.sharded = jax.jit(shard_map(
            _body, mesh=mesh,
            in_specs=(PartitionSpec("core"),) * len(in_names),
            out_specs=(PartitionSpec("core"),) * len(out_names),
            check_rep=False), keep_unused=True)

    def global_args(self, in_maps):
        return [np.concatenate([in_maps[c][nm] for c in range(NCORES)], 0)
                for nm in self.in_names]

    def __call__(self, in_maps):
        out = self.sharded(*self.global_args(in_maps))
        self.jax.block_until_ready(out)
        return np.asarray(out[0])


def _get_runner(TC):
    r = _RUNNER_CACHE.get(TC)
    if r is None:
        r = _Runner(TC)
        _RUNNER_CACHE[TC] = r
    return r


def kernel(**inputs):
    TC, in_maps = _prep(**inputs)
    runner = _get_runner(TC)
    out = runner(in_maps)                    # [NCORES*NPC, N] bf16
    return np.asarray(out, dtype=np.float32)


# revision 39
# speedup vs baseline: 1.1467x; 1.0803x over previous
"""Trainium2 Bass kernel for nn_DGN (3x NNConv GNN + all-pairs L1 CBT).

Strategy (8 NeuronCores, SPMD):
  - Edges sorted by (dst, src), sharded so core c owns destination nodes
    [256c, 256(c+1)) as two 128-node chunks; per-(core,chunk) edge lists are
    padded to a uniform tile count so a single SPMD program serves all cores.
  - Per 128-edge tile: PE computes the edge-MLP z = ea' @ W' (bias folded via
    a ones-row), DVE/ACT fuse relu+multiply-by-gathered-source-features, and
    PE scatter-matmuls (one-hot lhsT) accumulate the per-node mean and the
    i-contraction in a wide [n, out*in] PSUM accumulator, reduced at chunk
    end.  Root-weight term is injected as one extra matmul into its own PSUM.
  - Source-feature gather runs as a SWDGE dma_gather straight from the
    AllGather output: h slices are stored into 128-wide (256B-aligned) rows
    so the collective result doubles as the gather table, and the next
    layer's h-independent edge-MLP tiles (z matmul + relu) are emitted
    before each collective so PE/ACT hide its ~50us latency.
  - CBT: per 4-row batch of local output rows, one DVE tensor_scalar
    (subtract, max 0) against a 4x-replicated transposed-h tile produces
    relu(h[j,k]-h[i,k]) for all j,k; a selection matmul on PE sums over k
    into a [128, 2048] PSUM block accumulated over 32 batches, using
    sum|d| = 2*sum relu(d) - (R_j - R_i).

Perf notes (axon tunnel, 2026-08-08): per-exec wall time is dominated by a
~75 ms PJRT-over-axon dispatch floor plus input upload (~6 ms per array +
~160 MB/s).  So all large constant tables (the one-hot scatter matrix S,
selection/identity matrices, replicated gather indices) are built on-device
from a few KB of packed indices, inputs ship as three per-dtype blobs, the
zero output buffers are never uploaded (the custom call allocates outputs
fresh; every element of d_out is written), and the output is bf16.
"""

import numpy as np
import ml_dtypes

import concourse.bass as bass
import concourse.bacc as bacc
import concourse.tile as tile
import concourse.mybir as mybir

BF16 = mybir.dt.bfloat16
F32 = mybir.dt.float32
I16 = mybir.dt.int16

N = 2048
E = 65536
NV = 6
C = 32
NCORES = 8
NPC = N // NCORES      # nodes per core = 256
CHUNK = 128            # node chunk (PSUM partition dim)
Op = mybir.AluOpType

_RUNNER_CACHE = {}


# --------------------------------------------------------------------------
# host-side prep
# --------------------------------------------------------------------------

def _bf16(a):
    return np.asarray(a, dtype=np.float32).astype(ml_dtypes.bfloat16)


def _permute_w(Wf, b):
    """[NV, in*out] + [in*out] -> [7, in*out] with columns re-ordered from
    (i-major) i*out+o to (o-major) o*in+i, bias folded as last row."""
    in_c = Wf.shape[1] // C
    Wb = np.concatenate([Wf, b[None, :]], axis=0)  # [7, in*out]
    cols = Wb.reshape(NV + 1, in_c, C)             # [7, i, o]
    return np.transpose(cols, (0, 2, 1)).reshape(NV + 1, in_c * C)  # (o, i)


def _layout(TC):
    """Byte offsets of every tensor inside the three per-dtype blobs."""
    Tt = 2 * TC
    Ep = Tt * 128
    bb = dict()  # bf16 blob: name -> (offset_elems, shape)
    off = 0
    for name, shape in [("eaT7", (7, Ep)), ("W1p", (7, C)),
                        ("W2p", (7, C * C)), ("W3p", (7, C * C)),
                        ("idx", (16, Tt * 8))]:  # i16 bits carried as bf16
        bb[name] = (off, shape)
        off += shape[0] * shape[1]
    nb_b = off
    bf = dict()  # f32 blob
    off = 0
    for name, shape in [("xsrc", (128, Tt)), ("xT2", (2, NPC)),
                        ("invd", (128, 2)), ("r1", (2, C)),
                        ("r2", (C + 1, C)), ("r3", (C + 1, C)),
                        ("dstloc", (128, Tt)), ("colsel", (128, 1))]:
        bf[name] = (off, shape)
        off += shape[0] * shape[1]
    nb_f = off
    # f32 section appended to the bf16 blob (element offsets shifted by
    # the bf16 section's length in f32 units; nb_b is even so 4-aligned)
    bf = {name: (off + nb_b // 2, shape) for name, (off, shape) in bf.items()}
    return (bb, nb_b), (bf, nb_f)


def _prep(x, edge_attr, edge_index, W1, b1, root1, bias1, W2, b2, root2,
          bias2, W3, b3, root3, bias3):
    src = np.asarray(edge_index[0], dtype=np.int64)
    dst = np.asarray(edge_index[1], dtype=np.int64)
    x = np.asarray(x, dtype=np.float32)
    ea = np.asarray(edge_attr, dtype=np.float32)

    deg = np.bincount(dst, minlength=N).astype(np.float64)
    inv_deg = (1.0 / np.maximum(deg, 1.0)).astype(np.float32)

    order = np.lexsort((src, dst))
    s_srt, d_srt = src[order], dst[order]

    # per 128-dst-chunk edge index lists (into the original edge arrays)
    groups = []
    for g in range(N // CHUNK):          # 16 chunks
        sel = order[(d_srt >= g * CHUNK) & (d_srt < (g + 1) * CHUNK)]
        groups.append(sel)
    TC = max(1, max((len(g) + 127) // 128 for g in groups))
    Tt = 2 * TC
    Ep = Tt * 128

    (bb_l, nb_b), (bf_l, nb_f) = _layout(TC)

    shared_b = {
        "W1p": _bf16(np.concatenate([W1, b1[None, :]], 0)),
        "W2p": _bf16(_permute_w(W2, b2)),
        "W3p": _bf16(_permute_w(W3, b3)),
    }
    shared_f = {
        "r1": np.stack([root1[0], bias1], 0).astype(np.float32),
        "r2": np.concatenate([root2, bias2[None, :]], 0).astype(np.float32),
        "r3": np.concatenate([root3, bias3[None, :]], 0).astype(np.float32),
        "colsel": (124 + np.arange(128) // 32).astype(np.float32)[:, None],
    }

    in_maps = []
    for c in range(NCORES):
        eaT7 = np.zeros((7, Ep), np.float32)
        xsrc = np.zeros((128, Tt), np.float32)
        dstloc = np.full((128, Tt), 300.0, np.float32)  # 300 = no-match pad
        idx16 = np.zeros((16, Tt * 8), np.int16)
        for ch in range(2):
            g = groups[2 * c + ch]
            n = len(g)
            base = ch * TC * 128
            eaT7[:NV, base:base + n] = ea[g].T
            eaT7[NV, base:base + n] = 1.0
            gs = src[g]
            gd = dst[g]
            # edge-slot (p, t) layout: slot j of chunk ch -> p=j%128, t=j//128
            p = np.arange(n) % 128
            t = ch * TC + np.arange(n) // 128
            xsrc[p, t] = x[gs, 0]
            dstloc[p, t] = (gd - (2 * c + ch) * CHUNK).astype(np.float32)
            # gather indices, wrapped: idx j -> [j%16, j//16]
            ids = np.zeros(TC * 128, np.int16)
            ids[:n] = gs.astype(np.int16)
            idx16[:, ch * TC * 8:(ch + 1) * TC * 8] = \
                ids.reshape(TC * 8, 16).T
        xT2 = np.zeros((2, NPC), np.float32)
        xT2[0] = x[c * NPC:(c + 1) * NPC, 0]
        xT2[1] = 1.0
        invd = inv_deg[c * NPC:(c + 1) * NPC].reshape(2, 128).T.copy()

        per_b = {"eaT7": _bf16(eaT7), **shared_b,
                 "idx": np.ascontiguousarray(idx16).view(ml_dtypes.bfloat16)}
        per_f = {"xsrc": xsrc, "xT2": xT2, "invd": invd,
                 "dstloc": dstloc, **shared_f}
        blob_b = np.zeros((1, nb_b), ml_dtypes.bfloat16)
        for name, (off, shape) in bb_l.items():
            blob_b[0, off:off + shape[0] * shape[1]] = \
                np.ascontiguousarray(per_b[name]).reshape(-1)
        blob_f = np.zeros((1, nb_f), np.float32)
        base_f = nb_b // 2
        for name, (off, shape) in bf_l.items():
            blob_f[0, off - base_f:off - base_f + shape[0] * shape[1]] = \
                np.ascontiguousarray(per_f[name]).reshape(-1)
        blob = np.concatenate(
            [blob_b, blob_f.view(ml_dtypes.bfloat16)], axis=1)
        in_maps.append({"bb": blob})
    return TC, in_maps


# --------------------------------------------------------------------------
# device program
# --------------------------------------------------------------------------

def build_program(TC, reps=1, ablate=()):
    ablate = set(ablate)
    Tt = 2 * TC
    Ep = Tt * 128
    (bb_l, nb_b), (bf_l, nb_f) = _layout(TC)
    nc = bacc.Bacc("TRN2", target_bir_lowering=False, debug=False,
                   num_devices=NCORES)

    d_bb = nc.dram_tensor("bb", [1, nb_b + 2 * nb_f], BF16,
                          kind="ExternalInput")
    d_bf = d_bb.bitcast(F32)  # f32 section view (offsets pre-shifted)
    d_out = nc.dram_tensor("out", [NPC, N], BF16, kind="ExternalOutput")

    # layers 0/1: 128-wide rows so the AllGather output doubles as the
    # 256B-aligned dma_gather table (cols C:128 are never-read garbage)
    d_hsl = [nc.dram_tensor(f"hsl{l}", [NPC, 128], BF16) for l in range(2)]
    d_hsl.append(nc.dram_tensor("hsl2", [NPC, C], F32))
    d_hall = [nc.dram_tensor(f"hall{l}", [N, 128], BF16, addr_space="Shared")
              for l in range(2)]
    d_hall.append(nc.dram_tensor("hall2", [N, C], F32, addr_space="Shared"))

    RG = [list(range(NCORES))]

    def blob_ap(dram, layout, name):
        off, shape = layout[name]
        n = shape[0] * shape[1]
        return dram.ap()[0:1, off:off + n].rearrange(
            "a (p m) -> (a p) m", p=shape[0])

    with tile.TileContext(nc) as tc:
      for _rep in range(reps):
        with (
            tc.tile_pool(name="const", bufs=1) as cp,
            tc.tile_pool(name="hgp", bufs=2) as hgp,
            tc.tile_pool(name="msgp", bufs=6) as msgp,
            tc.tile_pool(name="wrp", bufs=TC + 2) as wrp,
            tc.tile_pool(name="tp", bufs=6) as tpp,
            tc.tile_pool(name="hcp", bufs=6) as hcp,
            tc.tile_pool(name="smf", bufs=6) as smf,
            tc.tile_pool(name="zp", bufs=2, space="PSUM") as zp,
            tc.tile_pool(name="aggp", bufs=1, space="PSUM") as aggp,
            tc.tile_pool(name="smp", bufs=1, space="PSUM") as smp,
        ):
            def bload(dram, layout, name, dtype, tag, shape=None):
                off, tshape = layout[name]
                shape = shape or tshape
                t = cp.tile(list(shape), dtype, tag=tag)
                nc.sync.dma_start(
                    out=t[0:tshape[0], :], in_=blob_ap(dram, layout, name))
                return t

            ea_sb = bload(d_bb, bb_l, "eaT7", BF16, "ea")
            w1_sb = bload(d_bb, bb_l, "W1p", BF16, "w1")
            w2_sb = bload(d_bb, bb_l, "W2p", BF16, "w2")
            w3_sb = bload(d_bb, bb_l, "W3p", BF16, "w3")
            xs_sb = bload(d_bf, bf_l, "xsrc", F32, "xs")
            xT2_sb = bload(d_bf, bf_l, "xT2", F32, "xT2")
            invd_sb = bload(d_bf, bf_l, "invd", F32, "invd")
            r1_sb = bload(d_bf, bf_l, "r1", F32, "r1")
            r2_sb = bload(d_bf, bf_l, "r2", F32, "r2")
            r3_sb = bload(d_bf, bf_l, "r3", F32, "r3")
            dst_sb = bload(d_bf, bf_l, "dstloc", F32, "dst")
            csel_sb = bload(d_bf, bf_l, "colsel", F32, "csel")
            # gather idx: i16 bits ride the bf16 blob; load wrapped
            # [16, Tt*8] then replicate to 128 rows (bitcast at use site)
            ix_sb = cp.tile([128, Tt * 8], BF16, tag="ix")
            nc.sync.dma_start(out=ix_sb[0:16, :],
                              in_=blob_ap(d_bb, bb_l, "idx"))
            nc.sync.dma_start(out=ix_sb[16:32, :], in_=ix_sb[0:16, :])
            nc.sync.dma_start(out=ix_sb[32:64, :], in_=ix_sb[0:32, :])
            nc.sync.dma_start(out=ix_sb[64:128, :], in_=ix_sb[0:64, :])

            # --------- on-device constant tables ---------
            iota128 = cp.tile([128, 128], F32, tag="iota128")
            nc.gpsimd.iota(iota128[:], pattern=[[1, 128]], base=0,
                           channel_multiplier=0,
                           allow_small_or_imprecise_dtypes=True)
            iotasel = cp.tile([128, 252], F32, tag="iotasel")
            nc.gpsimd.iota(iotasel[:], pattern=[[1, 252]], base=0,
                           channel_multiplier=0,
                           allow_small_or_imprecise_dtypes=True)
            idjp = cp.tile([128, 128], F32, tag="idjp")
            nc.gpsimd.iota(idjp[:], pattern=[[1, 128]], base=0,
                           channel_multiplier=-1,
                           allow_small_or_imprecise_dtypes=True)
            id32_sb = cp.tile([128, 128], F32, tag="id32")
            nc.vector.tensor_scalar(id32_sb[:], idjp[:], 0.0, None,
                                    Op.is_equal)
            sel_sb = cp.tile([128, 252], BF16, tag="sel")
            nc.vector.tensor_scalar(sel_sb[:], iotasel[:],
                                    csel_sb[:, 0:1], None, Op.is_equal)
            seln_sb = cp.tile([128, 128], F32, tag="seln")
            nc.vector.memset(seln_sb[0:32, :], -0.5)
            for qp in range(1, 4):
                nc.vector.memset(seln_sb[32 * qp:32 * (qp + 1), :], 0.0)
            # scatter one-hot S[e, n] = (dstloc[e,t] == n), built per tile
            S_sb = cp.tile([128, Ep], BF16, tag="S")
            for t in range(Tt):
                nc.vector.tensor_scalar(
                    S_sb[:, t * 128:(t + 1) * 128], iota128[:],
                    dst_sb[:, t:t + 1], None, Op.is_equal)

            # deterministic (never-read) padding columns of the gather tables
            zpad = cp.tile([128, 128 - C], BF16, tag="zpad")
            nc.vector.memset(zpad[:], 0.0)
            for l in range(2):
                for ch in range(2):
                    nc.sync.dma_start(
                        out=d_hsl[l][ch * 128:(ch + 1) * 128, C:128],
                        in_=zpad[:])

            hT1 = cp.tile([C + 1, NPC], F32, tag="hT1")
            hT2 = cp.tile([C + 1, NPC], F32, tag="hT2")
            hT3 = cp.tile([C, NPC], F32, tag="hT3")
            Rloc = cp.tile([128, 2], F32, tag="Rloc")
            nc.vector.memset(hT1[C:C + 1, :], 1.0)
            nc.vector.memset(hT2[C:C + 1, :], 1.0)

            # ---------------- layer 1 (in_c = 1) ----------------
            for ch in range(2):
                agg = aggp.tile([128, C], F32, tag="aggw")
                for t in range(TC):
                    gt = ch * TC + t
                    z1 = zp.tile([128, C], F32, tag="z")
                    nc.tensor.matmul(z1[:], ea_sb[:, gt * 128:(gt + 1) * 128],
                                     w1_sb[:], start=True, stop=True)
                    msg = msgp.tile([128, C], BF16, tag="msg")
                    nc.vector.tensor_scalar(
                        msg[:], z1[:], 0.0, xs_sb[:, gt:gt + 1],
                        Op.max, Op.mult)
                    nc.tensor.matmul(agg[:], S_sb[:, gt * 128:(gt + 1) * 128],
                                     msg[:], start=(t == 0), stop=(t == TC - 1))
                rtp = smp.tile([128, C], F32, tag="root")
                nc.tensor.matmul(rtp[:], xT2_sb[:, ch * 128:(ch + 1) * 128],
                                 r1_sb[:], start=True, stop=True)
                sm = smf.tile([128, C], F32, tag="sm")
                nc.vector.tensor_scalar(sm[:], agg[:],
                                        invd_sb[:, ch:ch + 1], None, Op.mult)
                hf_c = hcp.tile([128, C], F32, tag="hf")
                nc.vector.tensor_tensor(hf_c[:], sm[:], rtp[:], Op.add)
                nc.vector.tensor_scalar(hf_c[:], hf_c[:], 0.0, None, Op.max)
                h_c = hcp.tile([128, C], BF16, tag="hc")
                nc.scalar.copy(h_c[:], hf_c[:])
                tp = smp.tile([32, 128], F32, tag="tp")
                nc.tensor.transpose(tp[:], hf_c[:], id32_sb[:])
                nc.scalar.copy(hT1[0:C, ch * 128:(ch + 1) * 128], tp[:])
                nc.sync.dma_start(out=d_hsl[0][ch * 128:(ch + 1) * 128, 0:C],
                                  in_=h_c[:])
            def allgather(l):
                if "cc" in ablate:
                    nc.sync.dma_start(out=d_hall[l][0:NPC, :],
                                      in_=d_hsl[l].ap())
                else:
                    nc.gpsimd.collective_compute(
                        "AllGather", Op.bypass, replica_groups=RG,
                        ins=[d_hsl[l].ap()], outs=[d_hall[l].ap()])

            def phase_a(w_sb):
                """h-independent edge-MLP for chunk 0 of the next layer;
                emitted before the AllGather so PE/ACT overlap it."""
                tiles = []
                for t in range(TC):
                    z = zp.tile([128, C * C], F32, tag="z")
                    for q in range(2):
                        nc.tensor.matmul(
                            z[:, q * 512:(q + 1) * 512],
                            ea_sb[:, t * 128:(t + 1) * 128],
                            w_sb[:, q * 512:(q + 1) * 512],
                            start=True, stop=True)
                    wr = wrp.tile([128, C * C], BF16, tag="wr")
                    if t % 4 == 1:
                        nc.vector.tensor_scalar(wr[:], z[:], 0.0, None, Op.max)
                    else:
                        nc.scalar.activation(
                            wr[:], z[:], mybir.ActivationFunctionType.Relu)
                    tiles.append(wr)
                return tiles

            wrA = phase_a(w2_sb)
            allgather(0)

            # ---------------- layers 2 and 3 ----------------
            for li, (w_sb, r_sb, hTprev, hTcur) in enumerate(
                    [(w2_sb, r2_sb, hT1, hT2), (w3_sb, r3_sb, hT2, hT3)]):
                tab = d_hall[li]
                for ch in range(2):
                    hg = hgp.tile([128, TC, 128], BF16, tag="hg")
                    if "gather" in ablate:
                        # timing ablation: same bytes via plain DMA
                        done = 0
                        while done < TC:
                            nt = min(16, TC - done)
                            nc.sync.dma_start(
                                out=hg[:, done:done + nt, :],
                                in_=tab.ap()[0:nt * 128, :].rearrange(
                                    "(t p) e -> p t e", p=128))
                            done += nt
                    else:
                        # <=512 indices per dma_gather: one 4224-idx gather
                        # overflows the SWDGE descriptor ring (hang).
                        for g in range((TC * 128 + 511) // 512):
                            n_idx = min(512, TC * 128 - g * 512)
                            base = ch * TC * 8 + g * 32
                            nc.gpsimd.dma_gather(
                                out_ap=hg[:, g * 4:g * 4 + (n_idx + 127) // 128, :],
                                in_ap=tab.ap(),
                                idxs_ap=ix_sb[:, base:base + (n_idx + 15) // 16
                                              ].bitcast(I16),
                                num_idxs=n_idx, num_idxs_reg=n_idx,
                                elem_size=128)
                    aggw = aggp.tile([128, C * C], F32, tag="aggw")
                    for t in range(TC):
                        gt = ch * TC + t
                        if ch == 0:
                            wr = wrA[t]
                        else:
                            z = zp.tile([128, C * C], F32, tag="z")
                            for q in range(2):
                                nc.tensor.matmul(
                                    z[:, q * 512:(q + 1) * 512],
                                    ea_sb[:, gt * 128:(gt + 1) * 128],
                                    w_sb[:, q * 512:(q + 1) * 512],
                                    start=True, stop=True)
                            wr = wrp.tile([128, C * C], BF16, tag="wr")
                            if t % 4 == 1:
                                nc.vector.tensor_scalar(
                                    wr[:], z[:], 0.0, None, Op.max)
                            else:
                                nc.scalar.activation(
                                    wr[:], z[:],
                                    mybir.ActivationFunctionType.Relu)
                        tt = tpp.tile([128, C * C], BF16, tag="t")
                        t3 = tt[:].rearrange("p (o i) -> p o i", i=C)
                        hgb = hg[:, t:t + 1, 0:C].broadcast_to([128, C, C])
                        wr3 = wr[:].rearrange("p (o i) -> p o i", i=C)
                        nc.vector.tensor_tensor(t3, wr3, hgb, Op.mult)
                        for q in range(2):
                            nc.tensor.matmul(
                                aggw[:, q * 512:(q + 1) * 512],
                                S_sb[:, gt * 128:(gt + 1) * 128],
                                tt[:, q * 512:(q + 1) * 512],
                                start=(t == 0), stop=(t == TC - 1))
                    rtp = smp.tile([128, C], F32, tag="root")
                    nc.tensor.matmul(rtp[:],
                                     hTprev[:, ch * 128:(ch + 1) * 128],
                                     r_sb[:], start=True, stop=True)
                    red = smf.tile([128, C], F32, tag="red")
                    nc.vector.tensor_reduce(
                        red[:], aggw[:].rearrange("p (o i) -> p o i", i=C),
                        mybir.AxisListType.X, Op.add)
                    sm = smf.tile([128, C], F32, tag="sm")
                    nc.vector.tensor_scalar(sm[:], red[:],
                                            invd_sb[:, ch:ch + 1], None,
                                            Op.mult)
                    hf_c = hcp.tile([128, C], F32, tag="hf")
                    nc.vector.tensor_tensor(hf_c[:], sm[:], rtp[:], Op.add)
                    nc.vector.tensor_scalar(hf_c[:], hf_c[:], 0.0, None,
                                            Op.max)
                    if li == 1:
                        nc.vector.tensor_reduce(
                            Rloc[:, ch:ch + 1], hf_c[:],
                            mybir.AxisListType.X, Op.add)
                    tp = smp.tile([32, 128], F32, tag="tp")
                    nc.tensor.transpose(tp[:], hf_c[:], id32_sb[:])
                    nc.scalar.copy(hTcur[0:C, ch * 128:(ch + 1) * 128], tp[:])
                    if li == 0:
                        h_c = hcp.tile([128, C], BF16, tag="hc")
                        nc.scalar.copy(h_c[:], hf_c[:])
                        nc.sync.dma_start(
                            out=d_hsl[1][ch * 128:(ch + 1) * 128, 0:C],
                            in_=h_c[:])
                    else:
                        nc.sync.dma_start(
                            out=d_hsl[2][ch * 128:(ch + 1) * 128, :],
                            in_=hf_c[:])
                if li == 0:
                    wrA = phase_a(w3_sb)
                allgather(li + 1)

            # ---------------- CBT prep ----------------
            # f32 differencing: bf16 h here blows up per-entry relative
            # error on near-zero CBT entries (cancellation), keep f32
            hTrep = cp.tile([128, N], F32, tag="hTrep")
            h3f = cp.tile([128, 16, C], F32, tag="h3f")
            nc.sync.dma_start(
                out=h3f[:], in_=d_hall[2].ap().rearrange("(a b) c -> b a c",
                                                         a=16))
            for j in range(16):
                tp = smp.tile([32, 128], F32, tag="tp" if j % 2 else "root")
                nc.tensor.transpose(tp[:], h3f[:, j, :], id32_sb[:])
                nc.scalar.copy(hTrep[0:32, j * 128:(j + 1) * 128], tp[:])
            nc.sync.dma_start(out=hTrep[32:64, :], in_=hTrep[0:32, :])
            nc.sync.dma_start(out=hTrep[64:128, :], in_=hTrep[0:64, :])
            htcols = cp.tile([128, NPC // 4], F32, tag="htcols")
            htcolsn = cp.tile([128, NPC // 4], F32, tag="htcolsn")
            for s in range(4):
                nc.sync.dma_start(
                    out=htcols[s * 32:(s + 1) * 32, :],
                    in_=hT3[0:C, s::4])
            nc.vector.tensor_scalar(htcolsn[:], htcols[:], -1.0, None,
                                    Op.mult)

        # ---------------- CBT ----------------
        with (
            tc.tile_pool(name="ap_", bufs=5) as apool,
            tc.tile_pool(name="ob", bufs=4) as ob,
            tc.tile_pool(name="cbtp", bufs=2, space="PSUM") as cbtp,
        ):
            for ib in range(2):
                cps = cbtp.tile([128, N], F32, tag="cps")
                for b in range(32):
                    col = ib * 32 + b
                    a_b = apool.tile([128, N], BF16, tag="a")
                    if b % 7 >= 4:
                        nc.scalar.activation(
                            a_b[:], hTrep[:],
                            mybir.ActivationFunctionType.Relu,
                            bias=htcolsn[:, col:col + 1])
                    else:
                        nc.vector.tensor_scalar(
                            a_b[:], hTrep[:], htcols[:, col:col + 1], 0.0,
                            Op.subtract, Op.max)
                    lo = 124 - 4 * b
                    for q in range(4):
                        nc.tensor.matmul(
                            cps[:, q * 512:(q + 1) * 512],
                            sel_sb[:, lo:lo + 128],
                            a_b[:, q * 512:(q + 1) * 512],
                            start=(b == 0), stop=False)
                for q in range(4):
                    nc.tensor.matmul(
                        cps[:, q * 512:(q + 1) * 512], seln_sb[:],
                        hTrep[:, q * 512:(q + 1) * 512],
                        start=False, stop=True)
                for q in range(4):
                    ob_t = ob.tile([128, 512], BF16, tag="ob")
                    if q % 2 == 0:
                        nc.scalar.activation(
                            ob_t[:], cps[:, q * 512:(q + 1) * 512],
                            mybir.ActivationFunctionType.Identity,
                            bias=Rloc[:, ib:ib + 1], scale=2.0)
                    else:
                        nc.vector.tensor_scalar(
                            ob_t[:], cps[:, q * 512:(q + 1) * 512],
                            2.0, Rloc[:, ib:ib + 1], Op.mult, Op.add)
                    nc.sync.dma_start(
                        out=d_out[ib * 128:(ib + 1) * 128,
                                  q * 512:(q + 1) * 512],
                        in_=ob_t[:])

    nc.compile()
    return nc


# --------------------------------------------------------------------------
# runner: custom PJRT dispatch (no zero-output upload, 3 input arrays)
# --------------------------------------------------------------------------

class _Runner:
    def __init__(self, TC, reps=1):
        import jax
        from jax.sharding import Mesh, PartitionSpec
        from jax.experimental.shard_map import shard_map
        from concourse import bass2jax
        bass2jax.install_neuronx_cc_hook()

        self.jax = jax
        nc = build_program(TC, reps=reps)
        self.nc = nc
        partition_name = (nc.partition_id_tensor.name
                          if nc.partition_id_tensor else None)
        in_names, out_names, out_avals = [], [], []
        for alloc in nc.m.functions[0].allocations:
            if not isinstance(alloc, mybir.MemoryLocationSet):
                continue
            name = alloc.memorylocations[0].name
            if alloc.kind == "ExternalInput":
                if name != partition_name:
                    in_names.append(name)
            elif alloc.kind == "ExternalOutput":
                out_names.append(name)
                out_avals.append(jax.core.ShapedArray(
                    tuple(alloc.tensor_shape), mybir.dt.np(alloc.dtype)))
        self.in_names = in_names
        self.out_names = out_names
        all_in = list(in_names)
        if partition_name is not None:
            all_in.append(partition_name)

        def _body(*args):
            operands = list(args)
            if partition_name is not None:
                operands.append(bass2jax.partition_id_tensor())
            outs = bass2jax._bass_exec_p.bind(
                *operands, out_avals=tuple(out_avals),
                in_names=tuple(all_in), out_names=tuple(out_names),
                lowering_input_output_aliases=(),
                sim_require_finite=True, sim_require_nnan=True, nc=nc)
            return tuple(outs)

        devices = jax.devices()[:NCORES]
        mesh = Mesh(np.asarray(devices), ("core",))
        self.sharded = jax.jit(shard_map(
            _body, mesh=mesh,
            in_specs=(PartitionSpec("core"),) * len(in_names),
            out_specs=(PartitionSpec("core"),) * len(out_names),
            check_rep=False), keep_unused=True)

    def global_args(self, in_maps):
        return [np.concatenate([in_maps[c][nm] for c in range(NCORES)], 0)
                for nm in self.in_names]

    def __call__(self, in_maps):
        out = self.sharded(*self.global_args(in_maps))
        self.jax.block_until_ready(out)
        return np.asarray(out[0])


def _get_runner(TC):
    r = _RUNNER_CACHE.get(TC)
    if r is None:
        r = _Runner(TC)
        _RUNNER_CACHE[TC] = r
    return r


def kernel(**inputs):
    TC, in_maps = _prep(**inputs)
    runner = _get_runner(TC)
    out = runner(in_maps)                    # [NCORES*NPC, N] bf16
    return np.asarray(out, dtype=np.float32)


# revision 43
# speedup vs baseline: 1.1586x; 1.0104x over previous
"""Trainium2 Bass kernel for nn_DGN (3x NNConv GNN + all-pairs L1 CBT).

Strategy (8 NeuronCores, SPMD):
  - Edges sorted by (dst, src), sharded so core c owns destination nodes
    [256c, 256(c+1)) as two 128-node chunks; per-(core,chunk) edge lists are
    padded to a uniform tile count so a single SPMD program serves all cores.
  - Per 128-edge tile: PE computes the edge-MLP z = ea' @ W' (bias folded via
    a ones-row), DVE/ACT fuse relu+multiply-by-gathered-source-features, and
    PE scatter-matmuls (one-hot lhsT) accumulate the per-node mean and the
    i-contraction in a wide [n, out*in] PSUM accumulator, reduced at chunk
    end.  Root-weight term is injected as one extra matmul into its own PSUM.
  - Source-feature gather runs as a SWDGE dma_gather straight from the
    AllGather output: h slices are stored into 128-wide (256B-aligned) rows
    so the collective result doubles as the gather table, and the next
    layer's h-independent edge-MLP tiles (z matmul + relu) are emitted
    before each collective so PE/ACT hide its ~50us latency.
  - CBT: per 4-row batch of local output rows, one DVE tensor_scalar
    (subtract, max 0) against a 4x-replicated transposed-h tile produces
    relu(h[j,k]-h[i,k]) for all j,k; a selection matmul on PE sums over k
    into a [128, 2048] PSUM block accumulated over 32 batches, using
    sum|d| = 2*sum relu(d) - (R_j - R_i).

Perf notes (axon tunnel, 2026-08-08): per-exec wall time is dominated by a
~75 ms PJRT-over-axon dispatch floor plus input upload (~6 ms per array +
~160 MB/s).  So all large constant tables (the one-hot scatter matrix S,
selection/identity matrices, replicated gather indices) are built on-device
from a few KB of packed indices, inputs ship as three per-dtype blobs, the
zero output buffers are never uploaded (the custom call allocates outputs
fresh; every element of d_out is written), and the output is bf16.
"""

import numpy as np
import ml_dtypes

import concourse.bass as bass
import concourse.bacc as bacc
import concourse.tile as tile
import concourse.mybir as mybir

BF16 = mybir.dt.bfloat16
F32 = mybir.dt.float32
I16 = mybir.dt.int16

N = 2048
E = 65536
NV = 6
C = 32
NCORES = 8
NPC = N // NCORES      # nodes per core = 256
CHUNK = 128            # node chunk (PSUM partition dim)
Op = mybir.AluOpType

_RUNNER_CACHE = {}


# --------------------------------------------------------------------------
# host-side prep
# --------------------------------------------------------------------------

def _bf16(a):
    return np.asarray(a, dtype=np.float32).astype(ml_dtypes.bfloat16)


def _permute_w(Wf, b):
    """[NV, in*out] + [in*out] -> [7, in*out] with columns re-ordered from
    (i-major) i*out+o to (o-major) o*in+i, bias folded as last row."""
    in_c = Wf.shape[1] // C
    Wb = np.concatenate([Wf, b[None, :]], axis=0)  # [7, in*out]
    cols = Wb.reshape(NV + 1, in_c, C)             # [7, i, o]
    return np.transpose(cols, (0, 2, 1)).reshape(NV + 1, in_c * C)  # (o, i)


def _layout(TC):
    """Byte offsets of every tensor inside the three per-dtype blobs."""
    Tt = 2 * TC
    Ep = Tt * 128
    bb = dict()  # bf16 blob: name -> (offset_elems, shape)
    off = 0
    for name, shape in [("eaT7", (7, Ep)), ("W1p", (7, C)),
                        ("W2p", (7, C * C)), ("W3p", (7, C * C)),
                        ("idx", (16, Tt * 8)),   # i16 bits carried as bf16
                        ("dstloc", (128, Tt))]:  # ints 0..127/300, bf16-exact
        bb[name] = (off, shape)
        off += shape[0] * shape[1]
    nb_b = off
    bf = dict()  # f32 blob
    off = 0
    for name, shape in [("xsrc", (128, Tt)), ("xT2", (2, NPC)),
                        ("invd", (128, 2)), ("r1", (2, C)),
                        ("r2", (C + 1, C)), ("r3", (C + 1, C)),
                        ("colsel", (128, 1))]:
        bf[name] = (off, shape)
        off += shape[0] * shape[1]
    nb_f = off
    # f32 section appended to the bf16 blob (element offsets shifted by
    # the bf16 section's length in f32 units; nb_b is even so 4-aligned)
    bf = {name: (off + nb_b // 2, shape) for name, (off, shape) in bf.items()}
    return (bb, nb_b), (bf, nb_f)


def _prep(x, edge_attr, edge_index, W1, b1, root1, bias1, W2, b2, root2,
          bias2, W3, b3, root3, bias3):
    src = np.asarray(edge_index[0], dtype=np.int64)
    dst = np.asarray(edge_index[1], dtype=np.int64)
    x = np.asarray(x, dtype=np.float32)
    ea = np.asarray(edge_attr, dtype=np.float32)

    deg = np.bincount(dst, minlength=N).astype(np.float64)
    inv_deg = (1.0 / np.maximum(deg, 1.0)).astype(np.float32)

    order = np.lexsort((src, dst))
    s_srt, d_srt = src[order], dst[order]

    # per 128-dst-chunk edge index lists (into the original edge arrays)
    groups = []
    for g in range(N // CHUNK):          # 16 chunks
        sel = order[(d_srt >= g * CHUNK) & (d_srt < (g + 1) * CHUNK)]
        groups.append(sel)
    TC = max(1, max((len(g) + 127) // 128 for g in groups))
    Tt = 2 * TC
    Ep = Tt * 128

    (bb_l, nb_b), (bf_l, nb_f) = _layout(TC)

    shared_b = {
        "W1p": _bf16(np.concatenate([W1, b1[None, :]], 0)),
        "W2p": _bf16(_permute_w(W2, b2)),
        "W3p": _bf16(_permute_w(W3, b3)),
    }
    shared_f = {
        "r1": np.stack([root1[0], bias1], 0).astype(np.float32),
        "r2": np.concatenate([root2, bias2[None, :]], 0).astype(np.float32),
        "r3": np.concatenate([root3, bias3[None, :]], 0).astype(np.float32),
        "colsel": (124 + np.arange(128) // 32).astype(np.float32)[:, None],
    }

    in_maps = []
    for c in range(NCORES):
        eaT7 = np.zeros((7, Ep), np.float32)
        xsrc = np.zeros((128, Tt), np.float32)
        dstloc = np.full((128, Tt), 300.0, np.float32)  # 300 = no-match pad
        idx16 = np.zeros((16, Tt * 8), np.int16)
        for ch in range(2):
            g = groups[2 * c + ch]
            n = len(g)
            base = ch * TC * 128
            eaT7[:NV, base:base + n] = ea[g].T
            eaT7[NV, base:base + n] = 1.0
            gs = src[g]
            gd = dst[g]
            # edge-slot (p, t) layout: slot j of chunk ch -> p=j%128, t=j//128
            p = np.arange(n) % 128
            t = ch * TC + np.arange(n) // 128
            xsrc[p, t] = x[gs, 0]
            dstloc[p, t] = (gd - (2 * c + ch) * CHUNK).astype(np.float32)
            # gather indices, wrapped: idx j -> [j%16, j//16]
            ids = np.zeros(TC * 128, np.int16)
            ids[:n] = gs.astype(np.int16)
            idx16[:, ch * TC * 8:(ch + 1) * TC * 8] = \
                ids.reshape(TC * 8, 16).T
        xT2 = np.zeros((2, NPC), np.float32)
        xT2[0] = x[c * NPC:(c + 1) * NPC, 0]
        xT2[1] = 1.0
        invd = inv_deg[c * NPC:(c + 1) * NPC].reshape(2, 128).T.copy()

        per_b = {"eaT7": _bf16(eaT7), **shared_b, "dstloc": _bf16(dstloc),
                 "idx": np.ascontiguousarray(idx16).view(ml_dtypes.bfloat16)}
        per_f = {"xsrc": xsrc, "xT2": xT2, "invd": invd, **shared_f}
        blob_b = np.zeros((1, nb_b), ml_dtypes.bfloat16)
        for name, (off, shape) in bb_l.items():
            blob_b[0, off:off + shape[0] * shape[1]] = \
                np.ascontiguousarray(per_b[name]).reshape(-1)
        blob_f = np.zeros((1, nb_f), np.float32)
        base_f = nb_b // 2
        for name, (off, shape) in bf_l.items():
            blob_f[0, off - base_f:off - base_f + shape[0] * shape[1]] = \
                np.ascontiguousarray(per_f[name]).reshape(-1)
        blob = np.concatenate(
            [blob_b, blob_f.view(ml_dtypes.bfloat16)], axis=1)
        in_maps.append({"bb": blob})
    return TC, in_maps


# --------------------------------------------------------------------------
# device program
# --------------------------------------------------------------------------

def build_program(TC, reps=1, ablate=()):
    ablate = set(ablate)
    Tt = 2 * TC
    Ep = Tt * 128
    (bb_l, nb_b), (bf_l, nb_f) = _layout(TC)
    nc = bacc.Bacc("TRN2", target_bir_lowering=False, debug=False,
                   num_devices=NCORES)

    d_bb = nc.dram_tensor("bb", [1, nb_b + 2 * nb_f], BF16,
                          kind="ExternalInput")
    d_bf = d_bb.bitcast(F32)  # f32 section view (offsets pre-shifted)
    d_out = nc.dram_tensor("out", [NPC, N], BF16, kind="ExternalOutput")

    # layers 0/1: 128-wide rows so the AllGather output doubles as the
    # 256B-aligned dma_gather table (cols C:128 are never-read garbage)
    d_hsl = [nc.dram_tensor(f"hsl{l}", [NPC, 128], BF16) for l in range(2)]
    d_hsl.append(nc.dram_tensor("hsl2", [NPC, C], F32))
    d_hall = [nc.dram_tensor(f"hall{l}", [N, 128], BF16, addr_space="Shared")
              for l in range(2)]
    d_hall.append(nc.dram_tensor("hall2", [N, C], F32, addr_space="Shared"))

    RG = [list(range(NCORES))]

    def blob_ap(dram, layout, name):
        off, shape = layout[name]
        n = shape[0] * shape[1]
        return dram.ap()[0:1, off:off + n].rearrange(
            "a (p m) -> (a p) m", p=shape[0])

    with tile.TileContext(nc) as tc:
      for _rep in range(reps):
        with (
            tc.tile_pool(name="const", bufs=1) as cp,
            tc.tile_pool(name="hgp", bufs=2) as hgp,
            tc.tile_pool(name="msgp", bufs=6) as msgp,
            tc.tile_pool(name="wrp", bufs=TC + 2) as wrp,
            tc.tile_pool(name="tp", bufs=6) as tpp,
            tc.tile_pool(name="hcp", bufs=6) as hcp,
            tc.tile_pool(name="smf", bufs=6) as smf,
            tc.tile_pool(name="zp", bufs=2, space="PSUM") as zp,
            tc.tile_pool(name="aggp", bufs=1, space="PSUM") as aggp,
            tc.tile_pool(name="smp", bufs=1, space="PSUM") as smp,
        ):
            def bload(dram, layout, name, dtype, tag, shape=None):
                off, tshape = layout[name]
                shape = shape or tshape
                t = cp.tile(list(shape), dtype, tag=tag)
                nc.sync.dma_start(
                    out=t[0:tshape[0], :], in_=blob_ap(dram, layout, name))
                return t

            ea_sb = bload(d_bb, bb_l, "eaT7", BF16, "ea")
            w1_sb = bload(d_bb, bb_l, "W1p", BF16, "w1")
            w2_sb = bload(d_bb, bb_l, "W2p", BF16, "w2")
            w3_sb = bload(d_bb, bb_l, "W3p", BF16, "w3")
            xs_sb = bload(d_bf, bf_l, "xsrc", F32, "xs")
            xT2_sb = bload(d_bf, bf_l, "xT2", F32, "xT2")
            invd_sb = bload(d_bf, bf_l, "invd", F32, "invd")
            r1_sb = bload(d_bf, bf_l, "r1", F32, "r1")
            r2_sb = bload(d_bf, bf_l, "r2", F32, "r2")
            r3_sb = bload(d_bf, bf_l, "r3", F32, "r3")
            dstb_sb = bload(d_bb, bb_l, "dstloc", BF16, "dstb")
            dst_sb = cp.tile([128, Tt], F32, tag="dst")
            nc.scalar.copy(dst_sb[:], dstb_sb[:])  # exact: ints <= 300
            csel_sb = bload(d_bf, bf_l, "colsel", F32, "csel")
            # gather idx: i16 bits ride the bf16 blob; load wrapped
            # [16, Tt*8] then replicate to 128 rows (bitcast at use site)
            ix_sb = cp.tile([128, Tt * 8], BF16, tag="ix")
            nc.sync.dma_start(out=ix_sb[0:16, :],
                              in_=blob_ap(d_bb, bb_l, "idx"))
            nc.sync.dma_start(out=ix_sb[16:32, :], in_=ix_sb[0:16, :])
            nc.sync.dma_start(out=ix_sb[32:64, :], in_=ix_sb[0:32, :])
            nc.sync.dma_start(out=ix_sb[64:128, :], in_=ix_sb[0:64, :])

            # --------- on-device constant tables ---------
            iota128 = cp.tile([128, 128], F32, tag="iota128")
            nc.gpsimd.iota(iota128[:], pattern=[[1, 128]], base=0,
                           channel_multiplier=0,
                           allow_small_or_imprecise_dtypes=True)
            iotasel = cp.tile([128, 252], F32, tag="iotasel")
            nc.gpsimd.iota(iotasel[:], pattern=[[1, 252]], base=0,
                           channel_multiplier=0,
                           allow_small_or_imprecise_dtypes=True)
            idjp = cp.tile([128, 128], F32, tag="idjp")
            nc.gpsimd.iota(idjp[:], pattern=[[1, 128]], base=0,
                           channel_multiplier=-1,
                           allow_small_or_imprecise_dtypes=True)
            id32_sb = cp.tile([128, 128], F32, tag="id32")
            nc.vector.tensor_scalar(id32_sb[:], idjp[:], 0.0, None,
                                    Op.is_equal)
            sel_sb = cp.tile([128, 252], BF16, tag="sel")
            nc.vector.tensor_scalar(sel_sb[:], iotasel[:],
                                    csel_sb[:, 0:1], None, Op.is_equal)
            seln_sb = cp.tile([128, 128], F32, tag="seln")
            nc.vector.memset(seln_sb[0:32, :], -0.5)
            for qp in range(1, 4):
                nc.vector.memset(seln_sb[32 * qp:32 * (qp + 1), :], 0.0)
            # scatter one-hot S[e, n] = (dstloc[e,t] == n), built per tile
            S_sb = cp.tile([128, Ep], BF16, tag="S")
            for t in range(Tt):
                nc.vector.tensor_scalar(
                    S_sb[:, t * 128:(t + 1) * 128], iota128[:],
                    dst_sb[:, t:t + 1], None, Op.is_equal)

            # deterministic (never-read) padding columns of the gather tables
            zpad = cp.tile([128, 128 - C], BF16, tag="zpad")
            nc.vector.memset(zpad[:], 0.0)
            for l in range(2):
                for ch in range(2):
                    nc.sync.dma_start(
                        out=d_hsl[l][ch * 128:(ch + 1) * 128, C:128],
                        in_=zpad[:])

            hT1 = cp.tile([C + 1, NPC], F32, tag="hT1")
            hT2 = cp.tile([C + 1, NPC], F32, tag="hT2")
            hT3 = cp.tile([C, NPC], F32, tag="hT3")
            Rloc = cp.tile([128, 2], F32, tag="Rloc")
            nc.vector.memset(hT1[C:C + 1, :], 1.0)
            nc.vector.memset(hT2[C:C + 1, :], 1.0)

            # ---------------- layer 1 (in_c = 1) ----------------
            for ch in range(2):
                agg = aggp.tile([128, C], F32, tag="aggw")
                for t in range(TC):
                    gt = ch * TC + t
                    z1 = zp.tile([128, C], F32, tag="z")
                    nc.tensor.matmul(z1[:], ea_sb[:, gt * 128:(gt + 1) * 128],
                                     w1_sb[:], start=True, stop=True)
                    msg = msgp.tile([128, C], BF16, tag="msg")
                    nc.vector.tensor_scalar(
                        msg[:], z1[:], 0.0, xs_sb[:, gt:gt + 1],
                        Op.max, Op.mult)
                    nc.tensor.matmul(agg[:], S_sb[:, gt * 128:(gt + 1) * 128],
                                     msg[:], start=(t == 0), stop=(t == TC - 1))
                rtp = smp.tile([128, C], F32, tag="root")
                nc.tensor.matmul(rtp[:], xT2_sb[:, ch * 128:(ch + 1) * 128],
                                 r1_sb[:], start=True, stop=True)
                sm = smf.tile([128, C], F32, tag="sm")
                nc.vector.tensor_scalar(sm[:], agg[:],
                                        invd_sb[:, ch:ch + 1], None, Op.mult)
                hf_c = hcp.tile([128, C], F32, tag="hf")
                nc.vector.tensor_tensor(hf_c[:], sm[:], rtp[:], Op.add)
                nc.vector.tensor_scalar(hf_c[:], hf_c[:], 0.0, None, Op.max)
                h_c = hcp.tile([128, C], BF16, tag="hc")
                nc.scalar.copy(h_c[:], hf_c[:])
                tp = smp.tile([32, 128], F32, tag="tp")
                nc.tensor.transpose(tp[:], hf_c[:], id32_sb[:])
                nc.scalar.copy(hT1[0:C, ch * 128:(ch + 1) * 128], tp[:])
                nc.sync.dma_start(out=d_hsl[0][ch * 128:(ch + 1) * 128, 0:C],
                                  in_=h_c[:])
            def allgather(l):
                if "cc" in ablate:
                    nc.sync.dma_start(out=d_hall[l][0:NPC, :],
                                      in_=d_hsl[l].ap())
                else:
                    nc.gpsimd.collective_compute(
                        "AllGather", Op.bypass, replica_groups=RG,
                        ins=[d_hsl[l].ap()], outs=[d_hall[l].ap()])

            def phase_a(w_sb):
                """h-independent edge-MLP for chunk 0 of the next layer;
                emitted before the AllGather so PE/ACT overlap it."""
                tiles = []
                for t in range(TC):
                    z = zp.tile([128, C * C], F32, tag="z")
                    for q in range(2):
                        nc.tensor.matmul(
                            z[:, q * 512:(q + 1) * 512],
                            ea_sb[:, t * 128:(t + 1) * 128],
                            w_sb[:, q * 512:(q + 1) * 512],
                            start=True, stop=True)
                    wr = wrp.tile([128, C * C], BF16, tag="wr")
                    if t % 4 == 1:
                        nc.vector.tensor_scalar(wr[:], z[:], 0.0, None, Op.max)
                    else:
                        nc.scalar.activation(
                            wr[:], z[:], mybir.ActivationFunctionType.Relu)
                    tiles.append(wr)
                return tiles

            wrA = phase_a(w2_sb)
            allgather(0)

            # ---------------- layers 2 and 3 ----------------
            for li, (w_sb, r_sb, hTprev, hTcur) in enumerate(
                    [(w2_sb, r2_sb, hT1, hT2), (w3_sb, r3_sb, hT2, hT3)]):
                tab = d_hall[li]
                for ch in range(2):
                    hg = hgp.tile([128, TC, 128], BF16, tag="hg")
                    if "gather" in ablate:
                        # timing ablation: same bytes via plain DMA
                        done = 0
                        while done < TC:
                            nt = min(16, TC - done)
                            nc.sync.dma_start(
                                out=hg[:, done:done + nt, :],
                                in_=tab.ap()[0:nt * 128, :].rearrange(
                                    "(t p) e -> p t e", p=128))
                            done += nt
                    else:
                        # <=512 indices per dma_gather: one 4224-idx gather
                        # overflows the SWDGE descriptor ring (hang).
                        for g in range((TC * 128 + 511) // 512):
                            n_idx = min(512, TC * 128 - g * 512)
                            base = ch * TC * 8 + g * 32
                            nc.gpsimd.dma_gather(
                                out_ap=hg[:, g * 4:g * 4 + (n_idx + 127) // 128, :],
                                in_ap=tab.ap(),
                                idxs_ap=ix_sb[:, base:base + (n_idx + 15) // 16
                                              ].bitcast(I16),
                                num_idxs=n_idx, num_idxs_reg=n_idx,
                                elem_size=128)
                    aggw = aggp.tile([128, C * C], F32, tag="aggw")
                    for t in range(TC):
                        gt = ch * TC + t
                        if ch == 0:
                            wr = wrA[t]
                        else:
                            z = zp.tile([128, C * C], F32, tag="z")
                            for q in range(2):
                                nc.tensor.matmul(
                                    z[:, q * 512:(q + 1) * 512],
                                    ea_sb[:, gt * 128:(gt + 1) * 128],
                                    w_sb[:, q * 512:(q + 1) * 512],
                                    start=True, stop=True)
                            wr = wrp.tile([128, C * C], BF16, tag="wr")
                            if t % 4 == 1:
                                nc.vector.tensor_scalar(
                                    wr[:], z[:], 0.0, None, Op.max)
                            else:
                                nc.scalar.activation(
                                    wr[:], z[:],
                                    mybir.ActivationFunctionType.Relu)
                        tt = tpp.tile([128, C * C], BF16, tag="t")
                        t3 = tt[:].rearrange("p (o i) -> p o i", i=C)
                        hgb = hg[:, t:t + 1, 0:C].broadcast_to([128, C, C])
                        wr3 = wr[:].rearrange("p (o i) -> p o i", i=C)
                        nc.vector.tensor_tensor(t3, wr3, hgb, Op.mult)
                        for q in range(2):
                            nc.tensor.matmul(
                                aggw[:, q * 512:(q + 1) * 512],
                                S_sb[:, gt * 128:(gt + 1) * 128],
                                tt[:, q * 512:(q + 1) * 512],
                                start=(t == 0), stop=(t == TC - 1))
                    rtp = smp.tile([128, C], F32, tag="root")
                    nc.tensor.matmul(rtp[:],
                                     hTprev[:, ch * 128:(ch + 1) * 128],
                                     r_sb[:], start=True, stop=True)
                    red = smf.tile([128, C], F32, tag="red")
                    nc.vector.tensor_reduce(
                        red[:], aggw[:].rearrange("p (o i) -> p o i", i=C),
                        mybir.AxisListType.X, Op.add)
                    sm = smf.tile([128, C], F32, tag="sm")
                    nc.vector.tensor_scalar(sm[:], red[:],
                                            invd_sb[:, ch:ch + 1], None,
                                            Op.mult)
                    hf_c = hcp.tile([128, C], F32, tag="hf")
                    nc.vector.tensor_tensor(hf_c[:], sm[:], rtp[:], Op.add)
                    nc.vector.tensor_scalar(hf_c[:], hf_c[:], 0.0, None,
                                            Op.max)
                    if li == 1:
                        nc.vector.tensor_reduce(
                            Rloc[:, ch:ch + 1], hf_c[:],
                            mybir.AxisListType.X, Op.add)
                    tp = smp.tile([32, 128], F32, tag="tp")
                    nc.tensor.transpose(tp[:], hf_c[:], id32_sb[:])
                    nc.scalar.copy(hTcur[0:C, ch * 128:(ch + 1) * 128], tp[:])
                    if li == 0:
                        h_c = hcp.tile([128, C], BF16, tag="hc")
                        nc.scalar.copy(h_c[:], hf_c[:])
                        nc.sync.dma_start(
                            out=d_hsl[1][ch * 128:(ch + 1) * 128, 0:C],
                            in_=h_c[:])
                    else:
                        nc.sync.dma_start(
                            out=d_hsl[2][ch * 128:(ch + 1) * 128, :],
                            in_=hf_c[:])
                if li == 0:
                    wrA = phase_a(w3_sb)
                allgather(li + 1)

            # ---------------- CBT prep ----------------
            # f32 differencing: bf16 h here blows up per-entry relative
            # error on near-zero CBT entries (cancellation), keep f32
            hTrep = cp.tile([128, N], F32, tag="hTrep")
            h3f = cp.tile([128, 16, C], F32, tag="h3f")
            nc.sync.dma_start(
                out=h3f[:], in_=d_hall[2].ap().rearrange("(a b) c -> b a c",
                                                         a=16))
            for j in range(16):
                tp = smp.tile([32, 128], F32, tag="tp" if j % 2 else "root")
                nc.tensor.transpose(tp[:], h3f[:, j, :], id32_sb[:])
                nc.scalar.copy(hTrep[0:32, j * 128:(j + 1) * 128], tp[:])
            nc.sync.dma_start(out=hTrep[32:64, :], in_=hTrep[0:32, :])
            nc.sync.dma_start(out=hTrep[64:128, :], in_=hTrep[0:64, :])
            htcols = cp.tile([128, NPC // 4], F32, tag="htcols")
            htcolsn = cp.tile([128, NPC // 4], F32, tag="htcolsn")
            for s in range(4):
                nc.sync.dma_start(
                    out=htcols[s * 32:(s + 1) * 32, :],
                    in_=hT3[0:C, s::4])
            nc.vector.tensor_scalar(htcolsn[:], htcols[:], -1.0, None,
                                    Op.mult)

        # ---------------- CBT ----------------
        with (
            tc.tile_pool(name="ap_", bufs=5) as apool,
            tc.tile_pool(name="ob", bufs=4) as ob,
            tc.tile_pool(name="cbtp", bufs=2, space="PSUM") as cbtp,
        ):
            for ib in range(2):
                cps = cbtp.tile([128, N], F32, tag="cps")
                for b in range(32):
                    col = ib * 32 + b
                    a_b = apool.tile([128, N], BF16, tag="a")
                    if b % 7 >= 4:
                        nc.scalar.activation(
                            a_b[:], hTrep[:],
                            mybir.ActivationFunctionType.Relu,
                            bias=htcolsn[:, col:col + 1])
                    else:
                        nc.vector.tensor_scalar(
                            a_b[:], hTrep[:], htcols[:, col:col + 1], 0.0,
                            Op.subtract, Op.max)
                    lo = 124 - 4 * b
                    for q in range(4):
                        nc.tensor.matmul(
                            cps[:, q * 512:(q + 1) * 512],
                            sel_sb[:, lo:lo + 128],
                            a_b[:, q * 512:(q + 1) * 512],
                            start=(b == 0), stop=False)
                for q in range(4):
                    nc.tensor.matmul(
                        cps[:, q * 512:(q + 1) * 512], seln_sb[:],
                        hTrep[:, q * 512:(q + 1) * 512],
                        start=False, stop=True)
                for q in range(4):
                    ob_t = ob.tile([128, 512], BF16, tag="ob")
                    if q % 2 == 0:
                        nc.scalar.activation(
                            ob_t[:], cps[:, q * 512:(q + 1) * 512],
                            mybir.ActivationFunctionType.Identity,
                            bias=Rloc[:, ib:ib + 1], scale=2.0)
                    else:
                        nc.vector.tensor_scalar(
                            ob_t[:], cps[:, q * 512:(q + 1) * 512],
                            2.0, Rloc[:, ib:ib + 1], Op.mult, Op.add)
                    nc.sync.dma_start(
                        out=d_out[ib * 128:(ib + 1) * 128,
                                  q * 512:(q + 1) * 512],
                        in_=ob_t[:])

    nc.compile()
    return nc


# --------------------------------------------------------------------------
# runner: custom PJRT dispatch (no zero-output upload, 3 input arrays)
# --------------------------------------------------------------------------

class _Runner:
    def __init__(self, TC, reps=1):
        import jax
        from jax.sharding import Mesh, PartitionSpec
        from jax.experimental.shard_map import shard_map
        from concourse import bass2jax
        bass2jax.install_neuronx_cc_hook()

        self.jax = jax
        nc = build_program(TC, reps=reps)
        self.nc = nc
        partition_name = (nc.partition_id_tensor.name
                          if nc.partition_id_tensor else None)
        in_names, out_names, out_avals = [], [], []
        for alloc in nc.m.functions[0].allocations:
            if not isinstance(alloc, mybir.MemoryLocationSet):
                continue
            name = alloc.memorylocations[0].name
            if alloc.kind == "ExternalInput":
                if name != partition_name:
                    in_names.append(name)
            elif alloc.kind == "ExternalOutput":
                out_names.append(name)
                out_avals.append(jax.core.ShapedArray(
                    tuple(alloc.tensor_shape), mybir.dt.np(alloc.dtype)))
        self.in_names = in_names
        self.out_names = out_names
        all_in = list(in_names)
        if partition_name is not None:
            all_in.append(partition_name)

        def _body(*args):
            operands = list(args)
            if partition_name is not None:
                operands.append(bass2jax.partition_id_tensor())
            outs = bass2jax._bass_exec_p.bind(
                *operands, out_avals=tuple(out_avals),
                in_names=tuple(all_in), out_names=tuple(out_names),
                lowering_input_output_aliases=(),
                sim_require_finite=True, sim_require_nnan=True, nc=nc)
            return tuple(outs)

        devices = jax.devices()[:NCORES]
        mesh = Mesh(np.asarray(devices), ("core",))
        self.sharded = jax.jit(shard_map(
            _body, mesh=mesh,
            in_specs=(PartitionSpec("core"),) * len(in_names),
            out_specs=(PartitionSpec("core"),) * len(out_names),
            check_rep=False), keep_unused=True)

    def global_args(self, in_maps):
        return [np.concatenate([in_maps[c][nm] for c in range(NCORES)], 0)
                for nm in self.in_names]

    def __call__(self, in_maps):
        out = self.sharded(*self.global_args(in_maps))
        self.jax.block_until_ready(out)
        return np.asarray(out[0])


def _get_runner(TC):
    r = _RUNNER_CACHE.get(TC)
    if r is None:
        r = _Runner(TC)
        _RUNNER_CACHE[TC] = r
    return r


def kernel(**inputs):
    TC, in_maps = _prep(**inputs)
    runner = _get_runner(TC)
    out = runner(in_maps)                    # [NCORES*NPC, N] bf16
    return np.asarray(out, dtype=np.float32)
